# revision 30
# baseline (speedup 1.0000x reference)
"""Multi-head attention (B=4, S=2048, D=1024, H=16) on 8 TRN2 NeuronCores.

Sharding: core c handles batch b=c//2 and head-group g=c%2 (8 heads, 512 of
the 1024 model dims).  Wq/Wk/Wv column-parallel, Wo row-parallel; the two
head-group partial outputs per batch are summed on the host (no collectives).

Per-core dataflow (bf16 matmuls, fp32 PSUM accumulate):
  phase 1: Q.T = (Wq/8) @ x.T   [512,2048]   per head-pair tiles [128,512]
           K.T = Wk @ x.T       [512,2048]
           V   = x @ Wv.T       stored head-interleaved with a ones column:
                                [128, 4, 8*65]
  phase 2 (per 512-wide q-block, per head-PAIR p):
           per k-tile unit: two row-tiled concurrent matmuls (K=64 each,
             array rows 0-63 / 64-127) -> scores.T for both heads into one
             [128,2,512] PSUM tile (2 banks)
           ONE exp on ACT over [128,2,w] (both heads)
           causal diagonal chunks masked in-place by gpsimd.affine_select
           per head: raw[65,512] += [V_h|1].T @ expT  (row 64 = denominator)
           normalize: DVE reciprocal_approx_fast on raw[64], gpsimd
             partition broadcast, DVE multiply -> attnT bf16
  phase 3 (per q-block): outT += Wo_g.T.T @ attnT_cat -> [1024,2048] partial
Host: out[b] = (partial_g0 + partial_g1).T + bo
"""

import numpy as np
import ml_dtypes
from contextlib import ExitStack

B = 4
S = 2048
D = 1024
H = 16
DK = 64
G = 2                 # head groups
HL = H // G           # heads per core = 8
DL = D // G           # local head dims = 512
QB = 512              # q-block width
CH = 128              # chunk / k-tile width
NKT = S // CH         # 16 k-tiles
NQB = S // QB         # 4 q-blocks
NCORES = 8
NPAIR = HL // 2       # 4 head pairs per core


def _bf16(x):
    return np.ascontiguousarray(x, dtype=np.float32).astype(ml_dtypes.bfloat16)


def _plan_from_mask(m):
    """m: [S, S] bool, True = masked (scores[q, k] masked).

    Returns (plans, patterns):
      plans[qb][kt] = None (skip) or (c0, nch, mixed) where mixed is a list of
        (rel_chunk, kind, val): kind 'affine' -> val = base offset for
        gpsimd.affine_select (valid iff q_global - k_global + 0 >= 0 with
        base = q0 - k0); kind 'pat' -> val = index into patterns.
      patterns: list of unique [128,128] float32 0/1 valid-masks (scoresT
        orientation: [k_partition, q_free]) for non-affine mixed chunks.
    """
    patterns = []
    pat_index = {}
    plans = []
    kk = np.arange(CH)
    for qb in range(NQB):
        row = []
        for kt in range(NKT):
            sub = m[qb * QB:(qb + 1) * QB, kt * CH:(kt + 1) * CH]  # [q, k]
            valid = (~sub).T  # [k, q] 128 x 512
            nchunks = QB // CH
            kinds = []
            for c in range(nchunks):
                ch = valid[:, c * CH:(c + 1) * CH]
                if ch.all():
                    kinds.append("full")
                elif not ch.any():
                    kinds.append("empty")
                else:
                    kinds.append("mixed")
            not_empty = [c for c in range(nchunks) if kinds[c] != "empty"]
            if not not_empty:
                row.append(None)
                continue
            c0, c1 = not_empty[0], not_empty[-1]
            mixed = []
            for c in range(c0, c1 + 1):
                if kinds[c] == "full":
                    continue
                pat = valid[:, c * CH:(c + 1) * CH]
                # affine (causal) check: valid[k, q] == (q0 + q >= k0 + k)
                q0 = qb * QB + c * CH
                k0 = kt * CH
                base = q0 - k0
                aff = (base + kk[None, :] - kk[:, None]) >= 0
                if (pat == aff).all():
                    mixed.append((c - c0, "affine", base))
                else:
                    key = pat.tobytes()
                    if key not in pat_index:
                        pat_index[key] = len(patterns)
                        patterns.append(pat.astype(np.float32))
                    mixed.append((c - c0, "pat", pat_index[key]))
            row.append((c0, c1 - c0 + 1, mixed))
        plans.append(row)
    return plans, patterns


def _build(plans, n_patterns, guard_empty_rows, has_bias):
    import concourse.bacc as bacc
    import concourse.tile as tile
    from concourse import mybir

    F32 = mybir.dt.float32
    BF16 = mybir.dt.bfloat16
    AF = mybir.ActivationFunctionType
    GE = mybir.AluOpType.is_ge

    nc = bacc.Bacc("TRN2", target_bir_lowering=False, debug=False)

    xq = nc.dram_tensor("xq_t", [D, S], BF16, kind="ExternalInput")
    xk = nc.dram_tensor("xk_t", [D, S], BF16, kind="ExternalInput")
    xv = nc.dram_tensor("xv_t", [D, S], BF16, kind="ExternalInput")
    wq = nc.dram_tensor("wq_t", [D, DL], BF16, kind="ExternalInput")
    wk = nc.dram_tensor("wk_t", [D, DL], BF16, kind="ExternalInput")
    wv = nc.dram_tensor("wv_t", [D, DL], BF16, kind="ExternalInput")
    wo = nc.dram_tensor("wo_t", [DL, D], BF16, kind="ExternalInput")
    bq = nc.dram_tensor("bq8", [1, DL], BF16, kind="ExternalInput")
    bk = nc.dram_tensor("bk", [1, DL], BF16, kind="ExternalInput")
    bv = nc.dram_tensor("bv", [1, DL], BF16, kind="ExternalInput")
    onesr = nc.dram_tensor("ones_row", [1, QB], BF16, kind="ExternalInput")
    if n_patterns:
        maskp = nc.dram_tensor("maskp", [CH, n_patterns * CH], BF16,
                               kind="ExternalInput")
    outT = nc.dram_tensor("outT", [D, S], F32, kind="ExternalOutput")
    import os as _os
    DBG = bool(_os.environ.get("BASS_MHA_DEBUG"))
    if DBG:
        qt_dbg = nc.dram_tensor("qt_dbg", [CH, QB], BF16, kind="ExternalOutput")
        kt_dbg = nc.dram_tensor("kt_dbg", [CH, QB], BF16, kind="ExternalOutput")
        vg_dbg = nc.dram_tensor("vg_dbg", [CH, HL * (DK + 1)], BF16, kind="ExternalOutput")
        ex_dbg = nc.dram_tensor("ex_dbg", [CH, 2 * QB], BF16, kind="ExternalOutput")
        att_dbg = nc.dram_tensor("att_dbg", [CH, (DL // CH) * QB], BF16, kind="ExternalOutput")

    MT = DL // CH      # 4 head-pair tiles
    NQU = S // QB      # 4 s-quarters
    NK = D // CH       # 8 contraction tiles

    with tile.TileContext(nc) as tc, ExitStack() as ctx:
        persist = ctx.enter_context(tc.tile_pool(name="persist", bufs=1))
        xin = ctx.enter_context(tc.tile_pool(name="xin", bufs=32))
        wt = ctx.enter_context(tc.tile_pool(name="wt", bufs=25))
        expp = ctx.enter_context(tc.tile_pool(name="expp", bufs=4))
        rawcp = ctx.enter_context(tc.tile_pool(name="rawcp", bufs=2))
        attp = ctx.enter_context(tc.tile_pool(name="attp", bufs=2))
        outp = ctx.enter_context(tc.tile_pool(name="outp", bufs=4))
        recp = ctx.enter_context(tc.tile_pool(name="recp", bufs=2))
        ps_mm = ctx.enter_context(tc.tile_pool(name="ps_mm", bufs=2, space="PSUM"))
        ps_sc = ctx.enter_context(tc.tile_pool(name="ps_sc", bufs=2, space="PSUM"))
        ps_raw = ctx.enter_context(tc.tile_pool(name="ps_raw", bufs=1, space="PSUM"))

        # per-(pair, quarter) projection output tiles: heads 2p / 2p+1 live on
        # partitions 0-63 / 64-127 -> row-tiled concurrent score matmuls
        qt_q = {(p, qu): persist.tile([CH, QB], BF16, name=f"qt_{p}_{qu}")
                for p in range(MT) for qu in range(NQU)}
        kt_q = {(p, qu): persist.tile([CH, QB], BF16, name=f"kt_{p}_{qu}")
                for p in range(MT) for qu in range(NQU)}
        v_g = [persist.tile([CH, NQU, HL * (DK + 1)], BF16, name=f"v_g{qu}")
               for qu in range(NQU)]
        wo_all = persist.tile([CH, MT, D], BF16)
        ones_sb = persist.tile([1, QB], BF16)
        bq_sb = persist.tile([1, DL], BF16)
        bk_sb = persist.tile([1, DL], BF16)
        bv_sb = persist.tile([1, DL], BF16)
        if n_patterns:
            mp_sb = persist.tile([CH, n_patterns, CH], BF16)

        nc.sync.dma_start(ones_sb[:], onesr.ap())
        nc.sync.dma_start(bq_sb[:], bq.ap())
        nc.sync.dma_start(bk_sb[:], bk.ap())
        nc.sync.dma_start(bv_sb[:], bv.ap())

        # ACT exp-table preload off the critical path + PE warm-up while the
        # first input DMAs land
        dum = persist.tile([1, 16], F32)
        wu_ps = ps_mm.tile([1, QB], F32, tag="mm")
        for _ in range(10):
            nc.tensor.matmul(wu_ps[:], ones_sb[0:1, 0:1], ones_sb[0:1, :],
                             start=True, stop=True)
        nc.scalar.activation(dum[:], wu_ps[0:1, 0:16], AF.Exp)

        def load_w(dram):
            tiles = []
            for kt in range(NK):
                wtile = wt.tile([CH, DL], BF16, tag="w")
                nc.sync.dma_start(wtile[:], dram.ap()[kt * CH:(kt + 1) * CH, :])
                tiles.append(wtile)
            return tiles

        def load_x(dram, qu):
            tiles = []
            for kt in range(NK):
                xt = xin.tile([CH, QB], BF16, tag="x")
                nc.sync.dma_start(
                    xt[:], dram.ap()[kt * CH:(kt + 1) * CH,
                                     qu * QB:(qu + 1) * QB])
                tiles.append(xt)
            return tiles

        def proj_qk_m(x_tiles, w_tiles, bias_sb, dst_map, qu, p, use_bias):
            ps = ps_mm.tile([CH, QB], F32, tag="mm")
            for kt in range(NK):
                nc.tensor.matmul(
                    ps[:], w_tiles[kt][:, p * CH:(p + 1) * CH],
                    x_tiles[kt][:], start=(kt == 0),
                    stop=(not use_bias and kt == NK - 1))
            if use_bias:
                nc.tensor.matmul(
                    ps[:], bias_sb[0:1, p * CH:(p + 1) * CH],
                    ones_sb[0:1, :], start=False, stop=True)
            nc.vector.tensor_copy(out=dst_map[(p, qu)][:], in_=ps[:])

        def proj_v_j(x_tiles, qu, j):
            ps = ps_mm.tile([CH, DL], F32, tag="mm")
            for kt in range(NK):
                nc.tensor.matmul(
                    ps[:], x_tiles[kt][:, j * CH:(j + 1) * CH],
                    wv_t[kt][:], start=(kt == 0),
                    stop=(not has_bias[2] and kt == NK - 1))
            if has_bias[2]:
                nc.tensor.matmul(
                    ps[:], ones_sb[0:1, 0:CH], bv_sb[0:1, :],
                    start=False, stop=True)
            nc.vector.tensor_copy(
                out=v_g[qu][:, j, :].rearrange(
                    "p (h c) -> p h c", c=DK + 1)[:, :, 0:DK],
                in_=ps[:].rearrange("p (h c) -> p h c", c=DK),
            )

        zero_r = nc.gpsimd.to_reg(0.0)

        att_tiles = {}

        def attention_qb(qb):
            att = attp.tile([CH, MT, QB], BF16, tag="att")
            att_tiles[qb] = att
            units = [(kt,) + plans[qb][kt] for kt in range(NKT)
                     if plans[qb][kt] is not None]
            units = [(kt, c0, nch * CH, mixed)
                     for (kt, c0, nch, mixed) in units]
            for p in range(NPAIR):
                raw = ps_raw.tile([DK + 1, 2, QB], F32, tag="raw")
                nu = len(units)
                for ui, (kt, c0, w, mixed) in enumerate(units):
                    o = c0 * CH
                    sc = ps_sc.tile([CH, 2, QB], F32, tag="sc")
                    ktile = kt_q[(p, kt // 4)]
                    qtile = qt_q[(p, qb)]
                    ksl = slice((kt % 4) * CH, (kt % 4 + 1) * CH)
                    # two row-tiled concurrent matmuls (array rows 0-63/64-127)
                    nc.tensor.matmul(sc[:, 0, 0:w], ktile[0:DK, ksl],
                                     qtile[0:DK, o:o + w],
                                     start=True, stop=True)
                    nc.tensor.matmul(sc[:, 1, 0:w], ktile[DK:CH, ksl],
                                     qtile[DK:CH, o:o + w],
                                     start=True, stop=True)
                    ex = expp.tile([CH, 2, QB], BF16, tag="exp")
                    nc.scalar.activation(ex[:, :, 0:w], sc[:, :, 0:w], AF.Exp)
                    for (rel, kind, val) in mixed:
                        cs = slice(rel * CH, (rel + 1) * CH)
                        if kind == "affine":
                            nc.gpsimd.affine_select(
                                out=ex[:, :, cs], in_=ex[:, :, cs],
                                pattern=[[0, 2], [1, CH]],
                                compare_op=GE, fill=zero_r,
                                base=val, channel_multiplier=-1)
                        else:
                            for e in range(2):
                                nc.vector.tensor_mul(
                                    ex[:, e, cs], ex[:, e, cs],
                                    mp_sb[:, val, :])
                    if DBG and qb == 0 and p == 0 and kt == 0:
                        nc.sync.dma_start(
                            ex_dbg.ap().rearrange("p (a b) -> p a b", b=QB),
                            ex[:])
                    for e in range(2):
                        h = 2 * p + e
                        nc.tensor.matmul(
                            raw[:, e, o:o + w],
                            v_g[kt // 4][:, kt % 4,
                                         h * (DK + 1):(h + 1) * (DK + 1)],
                            ex[:, e, 0:w],
                            start=(ui == 0), stop=(ui == nu - 1))
                # single copy frees the raw PSUM banks for the next pair;
                # normalization then runs off the critical path from SBUF
                # (the custom-DVE reciprocal also requires SBUF input)
                rawc = rawcp.tile([DK + 1, 2, QB], F32, tag="rawc")
                nc.vector.tensor_copy(out=rawc[:], in_=raw[:])
                for e in range(2):
                    # stage the denominator at base partition 0: the custom-DVE
                    # reciprocal mishandles inputs at a nonzero base partition
                    rec = recp.tile([1, QB], F32, tag="rec")
                    den = recp.tile([1, QB], F32, tag="den")
                    nc.vector.tensor_scalar_max(
                        den[:], rawc[DK:DK + 1, e, :], 1e-30)
                    nc.vector.reciprocal_approx_fast(rec[:], den[:])
                    recb = recp.tile([DK, QB], F32, tag="recb")
                    nc.gpsimd.partition_broadcast(recb[:], rec[:])
                    nc.vector.tensor_mul(
                        att[e * DK:(e + 1) * DK, p, :], rawc[0:DK, e, :],
                        recb[:])

            if DBG and qb == 0:
                nc.sync.dma_start(
                    att_dbg.ap().rearrange("p (a b) -> p a b", b=QB), att[:])

        def outproj_qb(qb):
            att = att_tiles[qb]
            for mo in range(D // CH):
                ps = ps_mm.tile([CH, QB], F32, tag="mm")
                for ct in range(MT):
                    nc.tensor.matmul(
                        ps[:], wo_all[:, ct, mo * CH:(mo + 1) * CH],
                        att[:, ct, :], start=(ct == 0), stop=(ct == MT - 1))
                ot = outp.tile([CH, QB], F32, tag="ot")
                nc.any.tensor_copy(out=ot[:], in_=ps[:])
                nc.sync.dma_start(
                    outT.ap()[mo * CH:(mo + 1) * CH, qb * QB:(qb + 1) * QB],
                    ot[:])

        # ---- interleaved emission: proj rounds feed PE while attention ----
        # is exp(ACT)-paced; the scheduler fills PE bubbles with proj work.
        wk_t = load_w(wk)
        wq_t = load_w(wq)
        wv_t = load_w(wv)

        def proj_round(qu):
            # interleave per head-pair tile so attention pair p can start
            # after ~(p+1)/4 of the round instead of after the full round
            xk_tiles = load_x(xk, qu)
            xq_tiles = load_x(xq, qu)
            xv_tiles = load_x(xv, qu)
            # ones column interleaves with V values at 65-element stride: all
            # v_g writers must be the SAME engine (DVE) — a DMA writing the
            # ones column races the V copies within shared SBUF lines.
            nc.vector.memset(
                v_g[qu][:].rearrange("p s (h c) -> p s h c",
                                     c=DK + 1)[:, :, :, DK:DK + 1], 1.0)
            for i in range(MT):
                proj_qk_m(xk_tiles, wk_t, bk_sb, kt_q, qu, i, has_bias[1])
                proj_qk_m(xq_tiles, wq_t, bq_sb, qt_q, qu, i, has_bias[0])
                proj_v_j(xv_tiles, qu, i)

        proj_round(0)
        if DBG:
            nc.sync.dma_start(qt_dbg.ap(), qt_q[(0, 0)][:])
            nc.sync.dma_start(kt_dbg.ap(), kt_q[(0, 0)][:])
            nc.sync.dma_start(vg_dbg.ap(), v_g[0][:, 0, :])
        # deferred bulk constants (needed from attention onward)
        if n_patterns:
            nc.sync.dma_start(mp_sb[:], maskp.ap().rearrange(
                "p (u f) -> p u f", f=CH))
        nc.sync.dma_start(wo_all[:], wo.ap().rearrange("(t p) m -> p t m", p=CH))
        attention_qb(0)
        proj_round(1)
        attention_qb(1)
        outproj_qb(0)
        proj_round(2)
        attention_qb(2)
        outproj_qb(1)
        proj_round(3)
        attention_qb(3)
        outproj_qb(2)
        outproj_qb(3)

    nc.compile()
    return nc


_CACHE = {}
LAST_RESULTS = None


def _install_ntff_shim():
    """Provide antenv.axon_hooks (NTFF profiling) when the image lacks it."""
    import sys, types, ctypes, contextlib
    if "antenv.axon_hooks" in sys.modules:
        return
    import antenv
    mod = types.ModuleType("antenv.axon_hooks")
    state = {"hook": None}
    mod.set_axon_ntff_profile_hook = lambda h: state.__setitem__("hook", h)
    mod.get_axon_ntff_profile_hook = lambda: state["hook"]
    sys.modules["antenv.axon_hooks"] = mod
    antenv.axon_hooks = mod
    try:
        lib = ctypes.CDLL("/opt/axon/libaxon_pjrt.so")
    except OSError:
        return
    if not hasattr(lib, "axon_start_nrt_profile"):
        return
    lib.axon_start_nrt_profile.argtypes = [
        ctypes.POINTER(ctypes.c_int64), ctypes.c_size_t]
    lib.axon_start_nrt_profile.restype = ctypes.c_int64
    lib.axon_stop_nrt_profile.argtypes = [ctypes.c_char_p]
    lib.axon_stop_nrt_profile.restype = ctypes.c_int64

    @contextlib.contextmanager
    def _hook(output_dir, device_ids):
        import jax
        jax.devices()
        if device_ids:
            ids = (ctypes.c_int64 * len(device_ids))(*device_ids)
            rc = lib.axon_start_nrt_profile(ids, len(device_ids))
        else:
            rc = lib.axon_start_nrt_profile(None, 0)
        if rc != 0:
            raise RuntimeError(f"axon_start_nrt_profile rc={rc}")
        try:
            yield
        finally:
            n = lib.axon_stop_nrt_profile(str(output_dir).encode())
            print(f"profile: {n} ntff file(s) in {output_dir}", file=sys.stderr)

    state["hook"] = _hook


def _get_nc(mask2d, has_bias):
    key = (hash(mask2d.tobytes()), has_bias)
    if key not in _CACHE:
        plans, patterns = _plan_from_mask(mask2d)
        # guard against fully-masked rows (reference maps softmax NaN -> 0)
        valid_any = (~mask2d).any(axis=1)
        guard = bool((~valid_any).any())
        _CACHE[key] = (_build(plans, len(patterns), guard, has_bias), patterns)
    return _CACHE[key]


def kernel(query, key, value, mask, Wq, bq, Wk, bk, Wv, bv, Wo, bo):
    from concourse.bass_utils import run_bass_kernel_spmd

    query = np.asarray(query, dtype=np.float32)
    key_ = np.asarray(key, dtype=np.float32)
    value = np.asarray(value, dtype=np.float32)
    mask2d = np.asarray(mask, dtype=bool).reshape(S, S)
    Wq = np.asarray(Wq, dtype=np.float32)
    Wk = np.asarray(Wk, dtype=np.float32)
    Wv = np.asarray(Wv, dtype=np.float32)
    Wo = np.asarray(Wo, dtype=np.float32)
    bq = np.asarray(bq, dtype=np.float32)
    bk = np.asarray(bk, dtype=np.float32)
    bv = np.asarray(bv, dtype=np.float32)
    bo = np.asarray(bo, dtype=np.float32)

    has_bias = (bool(bq.any()), bool(bk.any()), bool(bv.any()))
    nc, patterns = _get_nc(mask2d, has_bias)

    n_pat = len(patterns)
    if n_pat:
        mp = np.empty((CH, n_pat * CH), np.float32)
        for u, pat in enumerate(patterns):
            mp[:, u * CH:(u + 1) * CH] = pat
        mp = mp.astype(ml_dtypes.bfloat16)
    ones_row = np.ones((1, QB), ml_dtypes.bfloat16)

    in_maps = []
    for c in range(NCORES):
        b, g = divmod(c, 2)
        gsl = slice(DL * g, DL * (g + 1))
        m = {
            "xq_t": _bf16(query[b].T),
            "xk_t": _bf16(key_[b].T),
            "xv_t": _bf16(value[b].T),
            "wq_t": _bf16(Wq[gsl].T * 0.125),
            "wk_t": _bf16(Wk[gsl].T),
            "wv_t": _bf16(Wv[gsl].T),
            "wo_t": _bf16(Wo[:, gsl].T),
            "bq8": _bf16(bq[gsl].reshape(1, DL) * 0.125),
            "bk": _bf16(bk[gsl].reshape(1, DL)),
            "bv": _bf16(bv[gsl].reshape(1, DL)),
            "ones_row": ones_row,
        }
        if n_pat:
            m["maskp"] = mp
        in_maps.append(m)

    import os
    kwargs = {}
    if os.environ.get("BASS_MHA_TRACE"):
        _install_ntff_shim()
        tc_env = os.environ.get("BASS_MHA_TRACE_CORES", "0")
        cores = (list(range(NCORES)) if tc_env == "all"
                 else [int(x) for x in tc_env.split(",")])
        kwargs = dict(trace=True, trace_cores=cores)
    res = run_bass_kernel_spmd(nc, in_maps, core_ids=list(range(NCORES)), **kwargs)
    global LAST_RESULTS
    LAST_RESULTS = res

    out = np.empty((B, S, D), np.float32)
    for b in range(B):
        acc = res.results[2 * b]["outT"] + res.results[2 * b + 1]["outT"]
        out[b] = acc.T + bo[None, :]
    return out


# revision 31
# speedup vs baseline: 1.2064x; 1.2064x over previous
"""Multi-head attention (B=4, S=2048, D=1024, H=16) on 8 TRN2 NeuronCores.

Sharding: core c handles batch b=c//2 and head-group g=c%2 (8 heads, 512 of
the 1024 model dims).  Wq/Wk/Wv column-parallel, Wo row-parallel; the two
head-group partial outputs per batch are summed on the host (no collectives).

Per-core dataflow (bf16 matmuls, fp32 PSUM accumulate):
  phase 1: Q.T = (Wq/8) @ x.T   [512,2048]   per head-pair tiles [128,512]
           K.T = Wk @ x.T       [512,2048]
           V   = x @ Wv.T       stored head-interleaved with a ones column:
                                [128, 4, 8*65]
  phase 2 (per 512-wide q-block, per head-PAIR p):
           per k-tile unit: two row-tiled concurrent matmuls (K=64 each,
             array rows 0-63 / 64-127) -> scores.T for both heads into one
             [128,2,512] PSUM tile (2 banks)
           ONE exp on ACT over [128,2,w] (both heads)
           causal diagonal chunks masked in-place by gpsimd.affine_select
           per head: raw[65,512] += [V_h|1].T @ expT  (row 64 = denominator)
           normalize: DVE reciprocal_approx_fast on raw[64], gpsimd
             partition broadcast, DVE multiply -> attnT bf16
  phase 3 (per q-block): outT += Wo_g.T.T @ attnT_cat -> [1024,2048] partial
Host: out[b] = (partial_g0 + partial_g1).T + bo
"""

import numpy as np
import ml_dtypes
from contextlib import ExitStack

B = 4
S = 2048
D = 1024
H = 16
DK = 64
G = 2                 # head groups
HL = H // G           # heads per core = 8
DL = D // G           # local head dims = 512
QB = 512              # q-block width
CH = 128              # chunk / k-tile width
NKT = S // CH         # 16 k-tiles
NQB = S // QB         # 4 q-blocks
NCORES = 8
NPAIR = HL // 2       # 4 head pairs per core


def _bf16(x):
    return np.ascontiguousarray(x, dtype=np.float32).astype(ml_dtypes.bfloat16)


def _plan_from_mask(m):
    """m: [S, S] bool, True = masked (scores[q, k] masked).

    Returns (plans, patterns):
      plans[qb][kt] = None (skip) or (c0, nch, mixed) where mixed is a list of
        (rel_chunk, kind, val): kind 'affine' -> val = base offset for
        gpsimd.affine_select (valid iff q_global - k_global + 0 >= 0 with
        base = q0 - k0); kind 'pat' -> val = index into patterns.
      patterns: list of unique [128,128] float32 0/1 valid-masks (scoresT
        orientation: [k_partition, q_free]) for non-affine mixed chunks.
    """
    patterns = []
    pat_index = {}
    plans = []
    kk = np.arange(CH)
    for qb in range(NQB):
        row = []
        for kt in range(NKT):
            sub = m[qb * QB:(qb + 1) * QB, kt * CH:(kt + 1) * CH]  # [q, k]
            valid = (~sub).T  # [k, q] 128 x 512
            nchunks = QB // CH
            kinds = []
            for c in range(nchunks):
                ch = valid[:, c * CH:(c + 1) * CH]
                if ch.all():
                    kinds.append("full")
                elif not ch.any():
                    kinds.append("empty")
                else:
                    kinds.append("mixed")
            not_empty = [c for c in range(nchunks) if kinds[c] != "empty"]
            if not not_empty:
                row.append(None)
                continue
            c0, c1 = not_empty[0], not_empty[-1]
            mixed = []
            for c in range(c0, c1 + 1):
                if kinds[c] == "full":
                    continue
                pat = valid[:, c * CH:(c + 1) * CH]
                # affine (causal) check: valid[k, q] == (q0 + q >= k0 + k)
                q0 = qb * QB + c * CH
                k0 = kt * CH
                base = q0 - k0
                aff = (base + kk[None, :] - kk[:, None]) >= 0
                if (pat == aff).all():
                    mixed.append((c - c0, "affine", base))
                else:
                    key = pat.tobytes()
                    if key not in pat_index:
                        pat_index[key] = len(patterns)
                        patterns.append(pat.astype(np.float32))
                    mixed.append((c - c0, "pat", pat_index[key]))
            row.append((c0, c1 - c0 + 1, mixed))
        plans.append(row)
    return plans, patterns


def _build(plans, n_patterns, guard_empty_rows, has_bias):
    import concourse.bacc as bacc
    import concourse.tile as tile
    from concourse import mybir

    F32 = mybir.dt.float32
    BF16 = mybir.dt.bfloat16
    AF = mybir.ActivationFunctionType
    GE = mybir.AluOpType.is_ge

    nc = bacc.Bacc("TRN2", target_bir_lowering=False, debug=False)

    xq = nc.dram_tensor("xq_t", [D, S], BF16, kind="ExternalInput")
    xk = nc.dram_tensor("xk_t", [D, S], BF16, kind="ExternalInput")
    xv = nc.dram_tensor("xv_t", [D, S], BF16, kind="ExternalInput")
    wq = nc.dram_tensor("wq_t", [D, DL], BF16, kind="ExternalInput")
    wk = nc.dram_tensor("wk_t", [D, DL], BF16, kind="ExternalInput")
    wv = nc.dram_tensor("wv_t", [D, DL], BF16, kind="ExternalInput")
    wo = nc.dram_tensor("wo_t", [DL, D], BF16, kind="ExternalInput")
    bq = nc.dram_tensor("bq8", [1, DL], BF16, kind="ExternalInput")
    bk = nc.dram_tensor("bk", [1, DL], BF16, kind="ExternalInput")
    bv = nc.dram_tensor("bv", [1, DL], BF16, kind="ExternalInput")
    onesr = nc.dram_tensor("ones_row", [1, QB], BF16, kind="ExternalInput")
    if n_patterns:
        maskp = nc.dram_tensor("maskp", [CH, n_patterns * CH], BF16,
                               kind="ExternalInput")
    outT = nc.dram_tensor("outT", [D, S], F32, kind="ExternalOutput")
    import os as _os
    DBG = bool(_os.environ.get("BASS_MHA_DEBUG"))
    if DBG:
        qt_dbg = nc.dram_tensor("qt_dbg", [CH, QB], BF16, kind="ExternalOutput")
        kt_dbg = nc.dram_tensor("kt_dbg", [CH, QB], BF16, kind="ExternalOutput")
        vg_dbg = nc.dram_tensor("vg_dbg", [CH, HL * (DK + 1)], BF16, kind="ExternalOutput")
        ex_dbg = nc.dram_tensor("ex_dbg", [CH, 2 * QB], BF16, kind="ExternalOutput")
        att_dbg = nc.dram_tensor("att_dbg", [CH, (DL // CH) * QB], BF16, kind="ExternalOutput")

    MT = DL // CH      # 4 head-pair tiles
    NQU = S // QB      # 4 s-quarters
    NK = D // CH       # 8 contraction tiles

    with tile.TileContext(nc) as tc, ExitStack() as ctx:
        persist = ctx.enter_context(tc.tile_pool(name="persist", bufs=1))
        xin = ctx.enter_context(tc.tile_pool(name="xin", bufs=32))
        wt = ctx.enter_context(tc.tile_pool(name="wt", bufs=25))
        expp = ctx.enter_context(tc.tile_pool(name="expp", bufs=4))
        rawcp = ctx.enter_context(tc.tile_pool(name="rawcp", bufs=2))
        attp = ctx.enter_context(tc.tile_pool(name="attp", bufs=2))
        outp = ctx.enter_context(tc.tile_pool(name="outp", bufs=4))
        recp = ctx.enter_context(tc.tile_pool(name="recp", bufs=2))
        ps_mm = ctx.enter_context(tc.tile_pool(name="ps_mm", bufs=2, space="PSUM"))
        ps_sc = ctx.enter_context(tc.tile_pool(name="ps_sc", bufs=2, space="PSUM"))
        ps_raw = ctx.enter_context(tc.tile_pool(name="ps_raw", bufs=1, space="PSUM"))

        # per-(pair, quarter) projection output tiles: heads 2p / 2p+1 live on
        # partitions 0-63 / 64-127 -> row-tiled concurrent score matmuls
        qt_q = {(p, qu): persist.tile([CH, QB], BF16, name=f"qt_{p}_{qu}")
                for p in range(MT) for qu in range(NQU)}
        kt_q = {(p, qu): persist.tile([CH, QB], BF16, name=f"kt_{p}_{qu}")
                for p in range(MT) for qu in range(NQU)}
        v_g = [persist.tile([CH, NQU, HL * (DK + 1)], BF16, name=f"v_g{qu}")
               for qu in range(NQU)]
        wo_all = persist.tile([CH, MT, D], BF16)
        ones_sb = persist.tile([1, QB], BF16)
        bq_sb = persist.tile([1, DL], BF16)
        bk_sb = persist.tile([1, DL], BF16)
        bv_sb = persist.tile([1, DL], BF16)
        if n_patterns:
            mp_sb = persist.tile([CH, n_patterns, CH], BF16)

        nc.sync.dma_start(ones_sb[:], onesr.ap())
        nc.sync.dma_start(bq_sb[:], bq.ap())
        nc.sync.dma_start(bk_sb[:], bk.ap())
        nc.sync.dma_start(bv_sb[:], bv.ap())

        # ACT exp-table preload off the critical path + PE warm-up while the
        # first input DMAs land
        dum = persist.tile([1, 16], F32)
        wu_ps = ps_mm.tile([1, QB], F32, tag="mm")
        for _ in range(10):
            nc.tensor.matmul(wu_ps[:], ones_sb[0:1, 0:1], ones_sb[0:1, :],
                             start=True, stop=True)
        nc.scalar.activation(dum[:], wu_ps[0:1, 0:16], AF.Exp)

        def load_w(dram):
            tiles = []
            for kt in range(NK):
                wtile = wt.tile([CH, DL], BF16, tag="w")
                nc.sync.dma_start(wtile[:], dram.ap()[kt * CH:(kt + 1) * CH, :])
                tiles.append(wtile)
            return tiles

        def load_x(dram, qu):
            tiles = []
            for kt in range(NK):
                xt = xin.tile([CH, QB], BF16, tag="x")
                nc.sync.dma_start(
                    xt[:], dram.ap()[kt * CH:(kt + 1) * CH,
                                     qu * QB:(qu + 1) * QB])
                tiles.append(xt)
            return tiles

        def proj_qk_m(x_tiles, w_tiles, bias_sb, dst_map, qu, p, use_bias):
            ps = ps_mm.tile([CH, QB], F32, tag="mm")
            for kt in range(NK):
                nc.tensor.matmul(
                    ps[:], w_tiles[kt][:, p * CH:(p + 1) * CH],
                    x_tiles[kt][:], start=(kt == 0),
                    stop=(not use_bias and kt == NK - 1))
            if use_bias:
                nc.tensor.matmul(
                    ps[:], bias_sb[0:1, p * CH:(p + 1) * CH],
                    ones_sb[0:1, :], start=False, stop=True)
            nc.any.tensor_copy(out=dst_map[(p, qu)][:], in_=ps[:])

        def proj_v_j(x_tiles, qu, j):
            ps = ps_mm.tile([CH, DL], F32, tag="mm")
            for kt in range(NK):
                nc.tensor.matmul(
                    ps[:], x_tiles[kt][:, j * CH:(j + 1) * CH],
                    wv_t[kt][:], start=(kt == 0),
                    stop=(not has_bias[2] and kt == NK - 1))
            if has_bias[2]:
                nc.tensor.matmul(
                    ps[:], ones_sb[0:1, 0:CH], bv_sb[0:1, :],
                    start=False, stop=True)
            nc.vector.tensor_copy(
                out=v_g[qu][:, j, :].rearrange(
                    "p (h c) -> p h c", c=DK + 1)[:, :, 0:DK],
                in_=ps[:].rearrange("p (h c) -> p h c", c=DK),
            )

        zero_r = nc.gpsimd.to_reg(0.0)

        att_tiles = {}

        def attention_qb(qb):
            att = attp.tile([CH, MT, QB], BF16, tag="att")
            att_tiles[qb] = att
            units = [(kt,) + plans[qb][kt] for kt in range(NKT)
                     if plans[qb][kt] is not None]
            units = [(kt, c0, nch * CH, mixed)
                     for (kt, c0, nch, mixed) in units]
            for p in range(NPAIR):
                raw = ps_raw.tile([DK + 1, 2, QB], F32, tag="raw")
                nu = len(units)
                for ui, (kt, c0, w, mixed) in enumerate(units):
                    o = c0 * CH
                    sc = ps_sc.tile([CH, 2, QB], F32, tag="sc")
                    ktile = kt_q[(p, kt // 4)]
                    qtile = qt_q[(p, qb)]
                    ksl = slice((kt % 4) * CH, (kt % 4 + 1) * CH)
                    # two row-tiled concurrent matmuls (array rows 0-63/64-127)
                    nc.tensor.matmul(sc[:, 0, 0:w], ktile[0:DK, ksl],
                                     qtile[0:DK, o:o + w],
                                     start=True, stop=True)
                    nc.tensor.matmul(sc[:, 1, 0:w], ktile[DK:CH, ksl],
                                     qtile[DK:CH, o:o + w],
                                     start=True, stop=True)
                    ex = expp.tile([CH, 2, QB], BF16, tag="exp")
                    nc.scalar.activation(ex[:, :, 0:w], sc[:, :, 0:w], AF.Exp)
                    for (rel, kind, val) in mixed:
                        cs = slice(rel * CH, (rel + 1) * CH)
                        if kind == "affine":
                            nc.gpsimd.affine_select(
                                out=ex[:, :, cs], in_=ex[:, :, cs],
                                pattern=[[0, 2], [1, CH]],
                                compare_op=GE, fill=zero_r,
                                base=val, channel_multiplier=-1)
                        else:
                            for e in range(2):
                                nc.vector.tensor_mul(
                                    ex[:, e, cs], ex[:, e, cs],
                                    mp_sb[:, val, :])
                    if DBG and qb == 0 and p == 0 and kt == 0:
                        nc.sync.dma_start(
                            ex_dbg.ap().rearrange("p (a b) -> p a b", b=QB),
                            ex[:])
                    for e in range(2):
                        h = 2 * p + e
                        nc.tensor.matmul(
                            raw[:, e, o:o + w],
                            v_g[kt // 4][:, kt % 4,
                                         h * (DK + 1):(h + 1) * (DK + 1)],
                            ex[:, e, 0:w],
                            start=(ui == 0), stop=(ui == nu - 1))
                # single copy frees the raw PSUM banks for the next pair;
                # normalization then runs off the critical path from SBUF
                # (the custom-DVE reciprocal also requires SBUF input)
                rawc = rawcp.tile([DK + 1, 2, QB], F32, tag="rawc")
                nc.vector.tensor_copy(out=rawc[:], in_=raw[:])
                for e in range(2):
                    # stage the denominator at base partition 0: the custom-DVE
                    # reciprocal mishandles inputs at a nonzero base partition
                    rec = recp.tile([1, QB], F32, tag="rec")
                    den = recp.tile([1, QB], F32, tag="den")
                    nc.vector.tensor_scalar_max(
                        den[:], rawc[DK:DK + 1, e, :], 1e-30)
                    nc.vector.reciprocal_approx_fast(rec[:], den[:])
                    recb = recp.tile([DK, QB], F32, tag="recb")
                    nc.gpsimd.partition_broadcast(recb[:], rec[:])
                    nc.vector.tensor_mul(
                        att[e * DK:(e + 1) * DK, p, :], rawc[0:DK, e, :],
                        recb[:])

            if DBG and qb == 0:
                nc.sync.dma_start(
                    att_dbg.ap().rearrange("p (a b) -> p a b", b=QB), att[:])

        def outproj_qb(qb):
            att = att_tiles[qb]
            for mo in range(D // CH):
                ps = ps_mm.tile([CH, QB], F32, tag="mm")
                for ct in range(MT):
                    nc.tensor.matmul(
                        ps[:], wo_all[:, ct, mo * CH:(mo + 1) * CH],
                        att[:, ct, :], start=(ct == 0), stop=(ct == MT - 1))
                ot = outp.tile([CH, QB], F32, tag="ot")
                nc.any.tensor_copy(out=ot[:], in_=ps[:])
                nc.sync.dma_start(
                    outT.ap()[mo * CH:(mo + 1) * CH, qb * QB:(qb + 1) * QB],
                    ot[:])

        # ---- interleaved emission: proj rounds feed PE while attention ----
        # is exp(ACT)-paced; the scheduler fills PE bubbles with proj work.
        wk_t = load_w(wk)
        wq_t = load_w(wq)
        wv_t = load_w(wv)

        def proj_round(qu):
            # interleave per head-pair tile so attention pair p can start
            # after ~(p+1)/4 of the round instead of after the full round
            xk_tiles = load_x(xk, qu)
            xq_tiles = load_x(xq, qu)
            xv_tiles = load_x(xv, qu)
            # ones column interleaves with V values at 65-element stride: all
            # v_g writers must be the SAME engine (DVE) — a DMA writing the
            # ones column races the V copies within shared SBUF lines.
            nc.vector.memset(
                v_g[qu][:].rearrange("p s (h c) -> p s h c",
                                     c=DK + 1)[:, :, :, DK:DK + 1], 1.0)
            for i in range(MT):
                proj_qk_m(xk_tiles, wk_t, bk_sb, kt_q, qu, i, has_bias[1])
                proj_qk_m(xq_tiles, wq_t, bq_sb, qt_q, qu, i, has_bias[0])
                proj_v_j(xv_tiles, qu, i)

        proj_round(0)
        if DBG:
            nc.sync.dma_start(qt_dbg.ap(), qt_q[(0, 0)][:])
            nc.sync.dma_start(kt_dbg.ap(), kt_q[(0, 0)][:])
            nc.sync.dma_start(vg_dbg.ap(), v_g[0][:, 0, :])
        # deferred bulk constants (needed from attention onward)
        if n_patterns:
            nc.sync.dma_start(mp_sb[:], maskp.ap().rearrange(
                "p (u f) -> p u f", f=CH))
        nc.sync.dma_start(wo_all[:], wo.ap().rearrange("(t p) m -> p t m", p=CH))
        attention_qb(0)
        proj_round(1)
        attention_qb(1)
        outproj_qb(0)
        proj_round(2)
        attention_qb(2)
        outproj_qb(1)
        proj_round(3)
        attention_qb(3)
        outproj_qb(2)
        outproj_qb(3)

    nc.compile()
    return nc


_CACHE = {}
LAST_RESULTS = None


def _install_ntff_shim():
    """Provide antenv.axon_hooks (NTFF profiling) when the image lacks it."""
    import sys, types, ctypes, contextlib
    if "antenv.axon_hooks" in sys.modules:
        return
    import antenv
    mod = types.ModuleType("antenv.axon_hooks")
    state = {"hook": None}
    mod.set_axon_ntff_profile_hook = lambda h: state.__setitem__("hook", h)
    mod.get_axon_ntff_profile_hook = lambda: state["hook"]
    sys.modules["antenv.axon_hooks"] = mod
    antenv.axon_hooks = mod
    try:
        lib = ctypes.CDLL("/opt/axon/libaxon_pjrt.so")
    except OSError:
        return
    if not hasattr(lib, "axon_start_nrt_profile"):
        return
    lib.axon_start_nrt_profile.argtypes = [
        ctypes.POINTER(ctypes.c_int64), ctypes.c_size_t]
    lib.axon_start_nrt_profile.restype = ctypes.c_int64
    lib.axon_stop_nrt_profile.argtypes = [ctypes.c_char_p]
    lib.axon_stop_nrt_profile.restype = ctypes.c_int64

    @contextlib.contextmanager
    def _hook(output_dir, device_ids):
        import jax
        jax.devices()
        if device_ids:
            ids = (ctypes.c_int64 * len(device_ids))(*device_ids)
            rc = lib.axon_start_nrt_profile(ids, len(device_ids))
        else:
            rc = lib.axon_start_nrt_profile(None, 0)
        if rc != 0:
            raise RuntimeError(f"axon_start_nrt_profile rc={rc}")
        try:
            yield
        finally:
            n = lib.axon_stop_nrt_profile(str(output_dir).encode())
            print(f"profile: {n} ntff file(s) in {output_dir}", file=sys.stderr)

    state["hook"] = _hook


def _get_nc(mask2d, has_bias):
    key = (hash(mask2d.tobytes()), has_bias)
    if key not in _CACHE:
        plans, patterns = _plan_from_mask(mask2d)
        # guard against fully-masked rows (reference maps softmax NaN -> 0)
        valid_any = (~mask2d).any(axis=1)
        guard = bool((~valid_any).any())
        _CACHE[key] = (_build(plans, len(patterns), guard, has_bias), patterns)
    return _CACHE[key]


def kernel(query, key, value, mask, Wq, bq, Wk, bk, Wv, bv, Wo, bo):
    from concourse.bass_utils import run_bass_kernel_spmd

    query = np.asarray(query, dtype=np.float32)
    key_ = np.asarray(key, dtype=np.float32)
    value = np.asarray(value, dtype=np.float32)
    mask2d = np.asarray(mask, dtype=bool).reshape(S, S)
    Wq = np.asarray(Wq, dtype=np.float32)
    Wk = np.asarray(Wk, dtype=np.float32)
    Wv = np.asarray(Wv, dtype=np.float32)
    Wo = np.asarray(Wo, dtype=np.float32)
    bq = np.asarray(bq, dtype=np.float32)
    bk = np.asarray(bk, dtype=np.float32)
    bv = np.asarray(bv, dtype=np.float32)
    bo = np.asarray(bo, dtype=np.float32)

    has_bias = (bool(bq.any()), bool(bk.any()), bool(bv.any()))
    nc, patterns = _get_nc(mask2d, has_bias)

    n_pat = len(patterns)
    if n_pat:
        mp = np.empty((CH, n_pat * CH), np.float32)
        for u, pat in enumerate(patterns):
            mp[:, u * CH:(u + 1) * CH] = pat
        mp = mp.astype(ml_dtypes.bfloat16)
    ones_row = np.ones((1, QB), ml_dtypes.bfloat16)

    in_maps = []
    for c in range(NCORES):
        b, g = divmod(c, 2)
        gsl = slice(DL * g, DL * (g + 1))
        m = {
            "xq_t": _bf16(query[b].T),
            "xk_t": _bf16(key_[b].T),
            "xv_t": _bf16(value[b].T),
            "wq_t": _bf16(Wq[gsl].T * 0.125),
            "wk_t": _bf16(Wk[gsl].T),
            "wv_t": _bf16(Wv[gsl].T),
            "wo_t": _bf16(Wo[:, gsl].T),
            "bq8": _bf16(bq[gsl].reshape(1, DL) * 0.125),
            "bk": _bf16(bk[gsl].reshape(1, DL)),
            "bv": _bf16(bv[gsl].reshape(1, DL)),
            "ones_row": ones_row,
        }
        if n_pat:
            m["maskp"] = mp
        in_maps.append(m)

    import os
    kwargs = {}
    if os.environ.get("BASS_MHA_TRACE"):
        _install_ntff_shim()
        tc_env = os.environ.get("BASS_MHA_TRACE_CORES", "0")
        cores = (list(range(NCORES)) if tc_env == "all"
                 else [int(x) for x in tc_env.split(",")])
        kwargs = dict(trace=True, trace_cores=cores)
    res = run_bass_kernel_spmd(nc, in_maps, core_ids=list(range(NCORES)), **kwargs)
    global LAST_RESULTS
    LAST_RESULTS = res

    out = np.empty((B, S, D), np.float32)
    for b in range(B):
        acc = res.results[2 * b]["outT"] + res.results[2 * b + 1]["outT"]
        out[b] = acc.T + bo[None, :]
    return out


# revision 32
# speedup vs baseline: 1.2484x; 1.0348x over previous
"""Multi-head attention (B=4, S=2048, D=1024, H=16) on 8 TRN2 NeuronCores.

Sharding: core c handles batch b=c//2 and head-group g=c%2 (8 heads, 512 of
the 1024 model dims).  Wq/Wk/Wv column-parallel, Wo row-parallel; the two
head-group partial outputs per batch are summed on the host (no collectives).

Per-core dataflow (bf16 matmuls, fp32 PSUM accumulate):
  phase 1: Q.T = (Wq/8) @ x.T   [512,2048]   per head-pair tiles [128,512]
           K.T = Wk @ x.T       [512,2048]
           V   = x @ Wv.T       stored head-interleaved with a ones column:
                                [128, 4, 8*65]
  phase 2 (per 512-wide q-block, per head-PAIR p):
           per k-tile unit: two row-tiled concurrent matmuls (K=64 each,
             array rows 0-63 / 64-127) -> scores.T for both heads into one
             [128,2,512] PSUM tile (2 banks)
           ONE exp on ACT over [128,2,w] (both heads)
           causal diagonal chunks masked in-place by gpsimd.affine_select
           per head: raw[65,512] += [V_h|1].T @ expT  (row 64 = denominator)
           normalize: DVE reciprocal_approx_fast on raw[64], gpsimd
             partition broadcast, DVE multiply -> attnT bf16
  phase 3 (per q-block): outT += Wo_g.T.T @ attnT_cat -> [1024,2048] partial
Host: out[b] = (partial_g0 + partial_g1).T + bo
"""

import numpy as np
import ml_dtypes
from contextlib import ExitStack

B = 4
S = 2048
D = 1024
H = 16
DK = 64
G = 2                 # head groups
HL = H // G           # heads per core = 8
DL = D // G           # local head dims = 512
QB = 512              # q-block width
CH = 128              # chunk / k-tile width
NKT = S // CH         # 16 k-tiles
NQB = S // QB         # 4 q-blocks
NCORES = 8
NPAIR = HL // 2       # 4 head pairs per core


def _bf16(x):
    return np.ascontiguousarray(x, dtype=np.float32).astype(ml_dtypes.bfloat16)


def _plan_from_mask(m):
    """m: [S, S] bool, True = masked (scores[q, k] masked).

    Returns (plans, patterns):
      plans[qb][kt] = None (skip) or (c0, nch, mixed) where mixed is a list of
        (rel_chunk, kind, val): kind 'affine' -> val = base offset for
        gpsimd.affine_select (valid iff q_global - k_global + 0 >= 0 with
        base = q0 - k0); kind 'pat' -> val = index into patterns.
      patterns: list of unique [128,128] float32 0/1 valid-masks (scoresT
        orientation: [k_partition, q_free]) for non-affine mixed chunks.
    """
    patterns = []
    pat_index = {}
    plans = []
    kk = np.arange(CH)
    for qb in range(NQB):
        row = []
        for kt in range(NKT):
            sub = m[qb * QB:(qb + 1) * QB, kt * CH:(kt + 1) * CH]  # [q, k]
            valid = (~sub).T  # [k, q] 128 x 512
            nchunks = QB // CH
            kinds = []
            for c in range(nchunks):
                ch = valid[:, c * CH:(c + 1) * CH]
                if ch.all():
                    kinds.append("full")
                elif not ch.any():
                    kinds.append("empty")
                else:
                    kinds.append("mixed")
            not_empty = [c for c in range(nchunks) if kinds[c] != "empty"]
            if not not_empty:
                row.append(None)
                continue
            c0, c1 = not_empty[0], not_empty[-1]
            mixed = []
            for c in range(c0, c1 + 1):
                if kinds[c] == "full":
                    continue
                pat = valid[:, c * CH:(c + 1) * CH]
                # affine (causal) check: valid[k, q] == (q0 + q >= k0 + k)
                q0 = qb * QB + c * CH
                k0 = kt * CH
                base = q0 - k0
                aff = (base + kk[None, :] - kk[:, None]) >= 0
                if (pat == aff).all():
                    mixed.append((c - c0, "affine", base))
                else:
                    key = pat.tobytes()
                    if key not in pat_index:
                        pat_index[key] = len(patterns)
                        patterns.append(pat.astype(np.float32))
                    mixed.append((c - c0, "pat", pat_index[key]))
            row.append((c0, c1 - c0 + 1, mixed))
        plans.append(row)
    return plans, patterns


def _build(plans, n_patterns, guard_empty_rows, has_bias):
    import concourse.bacc as bacc
    import concourse.tile as tile
    from concourse import mybir

    F32 = mybir.dt.float32
    BF16 = mybir.dt.bfloat16
    AF = mybir.ActivationFunctionType
    GE = mybir.AluOpType.is_ge

    nc = bacc.Bacc("TRN2", target_bir_lowering=False, debug=False)

    xq = nc.dram_tensor("xq_t", [D, S], BF16, kind="ExternalInput")
    xk = nc.dram_tensor("xk_t", [D, S], BF16, kind="ExternalInput")
    xv = nc.dram_tensor("xv_t", [D, S], BF16, kind="ExternalInput")
    wq = nc.dram_tensor("wq_t", [D, DL], BF16, kind="ExternalInput")
    wk = nc.dram_tensor("wk_t", [D, DL], BF16, kind="ExternalInput")
    wv = nc.dram_tensor("wv_t", [D, DL], BF16, kind="ExternalInput")
    wo = nc.dram_tensor("wo_t", [DL, D], BF16, kind="ExternalInput")
    bq = nc.dram_tensor("bq8", [1, DL], BF16, kind="ExternalInput")
    bk = nc.dram_tensor("bk", [1, DL], BF16, kind="ExternalInput")
    bv = nc.dram_tensor("bv", [1, DL], BF16, kind="ExternalInput")
    onesr = nc.dram_tensor("ones_row", [1, QB], BF16, kind="ExternalInput")
    if n_patterns:
        maskp = nc.dram_tensor("maskp", [CH, n_patterns * CH], BF16,
                               kind="ExternalInput")
    outT = nc.dram_tensor("outT", [D, S], F32, kind="ExternalOutput")
    import os as _os
    DBG = bool(_os.environ.get("BASS_MHA_DEBUG"))
    if DBG:
        qt_dbg = nc.dram_tensor("qt_dbg", [CH, QB], BF16, kind="ExternalOutput")
        kt_dbg = nc.dram_tensor("kt_dbg", [CH, QB], BF16, kind="ExternalOutput")
        vg_dbg = nc.dram_tensor("vg_dbg", [CH, HL * (DK + 1)], BF16, kind="ExternalOutput")
        ex_dbg = nc.dram_tensor("ex_dbg", [CH, 2 * QB], BF16, kind="ExternalOutput")
        att_dbg = nc.dram_tensor("att_dbg", [CH, (DL // CH) * QB], BF16, kind="ExternalOutput")

    MT = DL // CH      # 4 head-pair tiles
    NQU = S // QB      # 4 s-quarters
    NK = D // CH       # 8 contraction tiles

    with tile.TileContext(nc) as tc, ExitStack() as ctx:
        persist = ctx.enter_context(tc.tile_pool(name="persist", bufs=1))
        xin = ctx.enter_context(tc.tile_pool(name="xin", bufs=16))
        wt = ctx.enter_context(tc.tile_pool(name="wt", bufs=25))
        expp = ctx.enter_context(tc.tile_pool(name="expp", bufs=4))
        rawcp = ctx.enter_context(tc.tile_pool(name="rawcp", bufs=2))
        attp = ctx.enter_context(tc.tile_pool(name="attp", bufs=2))
        outp = ctx.enter_context(tc.tile_pool(name="outp", bufs=4))
        recp = ctx.enter_context(tc.tile_pool(name="recp", bufs=2))
        ps_mm = ctx.enter_context(tc.tile_pool(name="ps_mm", bufs=2, space="PSUM"))
        ps_sc = ctx.enter_context(tc.tile_pool(name="ps_sc", bufs=2, space="PSUM"))
        ps_raw = ctx.enter_context(tc.tile_pool(name="ps_raw", bufs=1, space="PSUM"))

        # per-(pair, quarter) projection output tiles: heads 2p / 2p+1 live on
        # partitions 0-63 / 64-127 -> row-tiled concurrent score matmuls
        qt_q = {(p, qu): persist.tile([CH, QB], BF16, name=f"qt_{p}_{qu}")
                for p in range(MT) for qu in range(NQU)}
        kt_q = {(p, qu): persist.tile([CH, QB], BF16, name=f"kt_{p}_{qu}")
                for p in range(MT) for qu in range(NQU)}
        v_g = [persist.tile([CH, NQU, HL * (DK + 1)], BF16, name=f"v_g{qu}")
               for qu in range(NQU)]
        wo_all = persist.tile([CH, MT, D], BF16)
        ones_sb = persist.tile([1, QB], BF16)
        bq_sb = persist.tile([1, DL], BF16)
        bk_sb = persist.tile([1, DL], BF16)
        bv_sb = persist.tile([1, DL], BF16)
        if n_patterns:
            mp_sb = persist.tile([CH, n_patterns, CH], BF16)

        nc.sync.dma_start(ones_sb[:], onesr.ap())
        nc.sync.dma_start(bq_sb[:], bq.ap())
        nc.sync.dma_start(bk_sb[:], bk.ap())
        nc.sync.dma_start(bv_sb[:], bv.ap())

        # ACT exp-table preload off the critical path + PE warm-up while the
        # first input DMAs land
        dum = persist.tile([1, 16], F32)
        wu_ps = ps_mm.tile([1, QB], F32, tag="mm")
        for _ in range(10):
            nc.tensor.matmul(wu_ps[:], ones_sb[0:1, 0:1], ones_sb[0:1, :],
                             start=True, stop=True)
        nc.scalar.activation(dum[:], wu_ps[0:1, 0:16], AF.Exp)

        def load_w(dram):
            tiles = []
            for kt in range(NK):
                wtile = wt.tile([CH, DL], BF16, tag="w")
                nc.sync.dma_start(wtile[:], dram.ap()[kt * CH:(kt + 1) * CH, :])
                tiles.append(wtile)
            return tiles

        def load_x(dram, qu):
            tiles = []
            for kt in range(NK):
                xt = xin.tile([CH, QB], BF16, tag="x")
                nc.sync.dma_start(
                    xt[:], dram.ap()[kt * CH:(kt + 1) * CH,
                                     qu * QB:(qu + 1) * QB])
                tiles.append(xt)
            return tiles

        def proj_qk_m(x_tiles, w_tiles, bias_sb, dst_map, qu, p, use_bias):
            ps = ps_mm.tile([CH, QB], F32, tag="mm")
            for kt in range(NK):
                nc.tensor.matmul(
                    ps[:], w_tiles[kt][:, p * CH:(p + 1) * CH],
                    x_tiles[kt][:], start=(kt == 0),
                    stop=(not use_bias and kt == NK - 1))
            if use_bias:
                nc.tensor.matmul(
                    ps[:], bias_sb[0:1, p * CH:(p + 1) * CH],
                    ones_sb[0:1, :], start=False, stop=True)
            nc.any.tensor_copy(out=dst_map[(p, qu)][:], in_=ps[:])

        def proj_v_j(x_tiles, qu, j):
            ps = ps_mm.tile([CH, DL], F32, tag="mm")
            for kt in range(NK):
                nc.tensor.matmul(
                    ps[:], x_tiles[kt][:, j * CH:(j + 1) * CH],
                    wv_t[kt][:], start=(kt == 0),
                    stop=(not has_bias[2] and kt == NK - 1))
            if has_bias[2]:
                nc.tensor.matmul(
                    ps[:], ones_sb[0:1, 0:CH], bv_sb[0:1, :],
                    start=False, stop=True)
            nc.vector.tensor_copy(
                out=v_g[qu][:, j, :].rearrange(
                    "p (h c) -> p h c", c=DK + 1)[:, :, 0:DK],
                in_=ps[:].rearrange("p (h c) -> p h c", c=DK),
            )

        zero_r = nc.gpsimd.to_reg(0.0)

        att_tiles = {}

        def attention_qb(qb):
            att = attp.tile([CH, MT, QB], BF16, tag="att")
            att_tiles[qb] = att
            units = [(kt,) + plans[qb][kt] for kt in range(NKT)
                     if plans[qb][kt] is not None]
            units = [(kt, c0, nch * CH, mixed)
                     for (kt, c0, nch, mixed) in units]
            for p in range(NPAIR):
                raw = ps_raw.tile([DK + 1, 2, QB], F32, tag="raw")
                nu = len(units)
                for ui, (kt, c0, w, mixed) in enumerate(units):
                    o = c0 * CH
                    sc = ps_sc.tile([CH, 2, QB], F32, tag="sc")
                    ktile = kt_q[(p, kt // 4)]
                    qtile = qt_q[(p, qb)]
                    ksl = slice((kt % 4) * CH, (kt % 4 + 1) * CH)
                    # two row-tiled concurrent matmuls (array rows 0-63/64-127)
                    nc.tensor.matmul(sc[:, 0, 0:w], ktile[0:DK, ksl],
                                     qtile[0:DK, o:o + w],
                                     start=True, stop=True)
                    nc.tensor.matmul(sc[:, 1, 0:w], ktile[DK:CH, ksl],
                                     qtile[DK:CH, o:o + w],
                                     start=True, stop=True)
                    ex = expp.tile([CH, 2, QB], BF16, tag="exp")
                    nc.scalar.activation(ex[:, :, 0:w], sc[:, :, 0:w], AF.Exp)
                    for (rel, kind, val) in mixed:
                        cs = slice(rel * CH, (rel + 1) * CH)
                        if kind == "affine":
                            nc.gpsimd.affine_select(
                                out=ex[:, :, cs], in_=ex[:, :, cs],
                                pattern=[[0, 2], [1, CH]],
                                compare_op=GE, fill=zero_r,
                                base=val, channel_multiplier=-1)
                        else:
                            for e in range(2):
                                nc.vector.tensor_mul(
                                    ex[:, e, cs], ex[:, e, cs],
                                    mp_sb[:, val, :])
                    if DBG and qb == 0 and p == 0 and kt == 0:
                        nc.sync.dma_start(
                            ex_dbg.ap().rearrange("p (a b) -> p a b", b=QB),
                            ex[:])
                    for e in range(2):
                        h = 2 * p + e
                        nc.tensor.matmul(
                            raw[:, e, o:o + w],
                            v_g[kt // 4][:, kt % 4,
                                         h * (DK + 1):(h + 1) * (DK + 1)],
                            ex[:, e, 0:w],
                            start=(ui == 0), stop=(ui == nu - 1))
                # single copy frees the raw PSUM banks for the next pair;
                # normalization then runs off the critical path from SBUF
                # (the custom-DVE reciprocal also requires SBUF input)
                rawc = rawcp.tile([DK + 1, 2, QB], F32, tag="rawc")
                nc.vector.tensor_copy(out=rawc[:], in_=raw[:])
                for e in range(2):
                    # stage the denominator at base partition 0: the custom-DVE
                    # reciprocal mishandles inputs at a nonzero base partition
                    rec = recp.tile([1, QB], F32, tag="rec")
                    den = recp.tile([1, QB], F32, tag="den")
                    nc.vector.tensor_scalar_max(
                        den[:], rawc[DK:DK + 1, e, :], 1e-30)
                    nc.vector.reciprocal_approx_fast(rec[:], den[:])
                    recb = recp.tile([DK, QB], F32, tag="recb")
                    nc.gpsimd.partition_broadcast(recb[:], rec[:])
                    nc.vector.tensor_mul(
                        att[e * DK:(e + 1) * DK, p, :], rawc[0:DK, e, :],
                        recb[:])

            if DBG and qb == 0:
                nc.sync.dma_start(
                    att_dbg.ap().rearrange("p (a b) -> p a b", b=QB), att[:])

        def outproj_qb(qb):
            att = att_tiles[qb]
            for mo in range(D // CH):
                ps = ps_mm.tile([CH, QB], F32, tag="mm")
                for ct in range(MT):
                    nc.tensor.matmul(
                        ps[:], wo_all[:, ct, mo * CH:(mo + 1) * CH],
                        att[:, ct, :], start=(ct == 0), stop=(ct == MT - 1))
                ot = outp.tile([CH, QB], F32, tag="ot")
                nc.any.tensor_copy(out=ot[:], in_=ps[:])
                nc.sync.dma_start(
                    outT.ap()[mo * CH:(mo + 1) * CH, qb * QB:(qb + 1) * QB],
                    ot[:])

        # ---- interleaved emission: proj rounds feed PE while attention ----
        # is exp(ACT)-paced; the scheduler fills PE bubbles with proj work.
        wk_t = load_w(wk)
        wq_t = load_w(wq)
        wv_t = load_w(wv)

        def proj_round(qu):
            xk_tiles = load_x(xk, qu)
            for i in range(MT):
                proj_qk_m(xk_tiles, wk_t, bk_sb, kt_q, qu, i, has_bias[1])
            xq_tiles = load_x(xq, qu)
            for i in range(MT):
                proj_qk_m(xq_tiles, wq_t, bq_sb, qt_q, qu, i, has_bias[0])
            xv_tiles = load_x(xv, qu)
            # ones column interleaves with V values at 65-element stride: all
            # v_g writers must be the SAME engine (DVE) — a DMA writing the
            # ones column races the V copies within shared SBUF lines.
            nc.vector.memset(
                v_g[qu][:].rearrange("p s (h c) -> p s h c",
                                     c=DK + 1)[:, :, :, DK:DK + 1], 1.0)
            for i in range(MT):
                proj_v_j(xv_tiles, qu, i)

        proj_round(0)
        if DBG:
            nc.sync.dma_start(qt_dbg.ap(), qt_q[(0, 0)][:])
            nc.sync.dma_start(kt_dbg.ap(), kt_q[(0, 0)][:])
            nc.sync.dma_start(vg_dbg.ap(), v_g[0][:, 0, :])
        # deferred bulk constants (needed from attention onward)
        if n_patterns:
            nc.sync.dma_start(mp_sb[:], maskp.ap().rearrange(
                "p (u f) -> p u f", f=CH))
        nc.sync.dma_start(wo_all[:], wo.ap().rearrange("(t p) m -> p t m", p=CH))
        attention_qb(0)
        proj_round(1)
        attention_qb(1)
        outproj_qb(0)
        proj_round(2)
        attention_qb(2)
        outproj_qb(1)
        proj_round(3)
        attention_qb(3)
        outproj_qb(2)
        outproj_qb(3)

    nc.compile()
    return nc


_CACHE = {}
LAST_RESULTS = None


def _install_ntff_shim():
    """Provide antenv.axon_hooks (NTFF profiling) when the image lacks it."""
    import sys, types, ctypes, contextlib
    if "antenv.axon_hooks" in sys.modules:
        return
    import antenv
    mod = types.ModuleType("antenv.axon_hooks")
    state = {"hook": None}
    mod.set_axon_ntff_profile_hook = lambda h: state.__setitem__("hook", h)
    mod.get_axon_ntff_profile_hook = lambda: state["hook"]
    sys.modules["antenv.axon_hooks"] = mod
    antenv.axon_hooks = mod
    try:
        lib = ctypes.CDLL("/opt/axon/libaxon_pjrt.so")
    except OSError:
        return
    if not hasattr(lib, "axon_start_nrt_profile"):
        return
    lib.axon_start_nrt_profile.argtypes = [
        ctypes.POINTER(ctypes.c_int64), ctypes.c_size_t]
    lib.axon_start_nrt_profile.restype = ctypes.c_int64
    lib.axon_stop_nrt_profile.argtypes = [ctypes.c_char_p]
    lib.axon_stop_nrt_profile.restype = ctypes.c_int64

    @contextlib.contextmanager
    def _hook(output_dir, device_ids):
        import jax
        jax.devices()
        if device_ids:
            ids = (ctypes.c_int64 * len(device_ids))(*device_ids)
            rc = lib.axon_start_nrt_profile(ids, len(device_ids))
        else:
            rc = lib.axon_start_nrt_profile(None, 0)
        if rc != 0:
            raise RuntimeError(f"axon_start_nrt_profile rc={rc}")
        try:
            yield
        finally:
            n = lib.axon_stop_nrt_profile(str(output_dir).encode())
            print(f"profile: {n} ntff file(s) in {output_dir}", file=sys.stderr)

    state["hook"] = _hook


def _get_nc(mask2d, has_bias):
    key = (hash(mask2d.tobytes()), has_bias)
    if key not in _CACHE:
        plans, patterns = _plan_from_mask(mask2d)
        # guard against fully-masked rows (reference maps softmax NaN -> 0)
        valid_any = (~mask2d).any(axis=1)
        guard = bool((~valid_any).any())
        _CACHE[key] = (_build(plans, len(patterns), guard, has_bias), patterns)
    return _CACHE[key]


def kernel(query, key, value, mask, Wq, bq, Wk, bk, Wv, bv, Wo, bo):
    from concourse.bass_utils import run_bass_kernel_spmd

    query = np.asarray(query, dtype=np.float32)
    key_ = np.asarray(key, dtype=np.float32)
    value = np.asarray(value, dtype=np.float32)
    mask2d = np.asarray(mask, dtype=bool).reshape(S, S)
    Wq = np.asarray(Wq, dtype=np.float32)
    Wk = np.asarray(Wk, dtype=np.float32)
    Wv = np.asarray(Wv, dtype=np.float32)
    Wo = np.asarray(Wo, dtype=np.float32)
    bq = np.asarray(bq, dtype=np.float32)
    bk = np.asarray(bk, dtype=np.float32)
    bv = np.asarray(bv, dtype=np.float32)
    bo = np.asarray(bo, dtype=np.float32)

    has_bias = (bool(bq.any()), bool(bk.any()), bool(bv.any()))
    nc, patterns = _get_nc(mask2d, has_bias)

    n_pat = len(patterns)
    if n_pat:
        mp = np.empty((CH, n_pat * CH), np.float32)
        for u, pat in enumerate(patterns):
            mp[:, u * CH:(u + 1) * CH] = pat
        mp = mp.astype(ml_dtypes.bfloat16)
    ones_row = np.ones((1, QB), ml_dtypes.bfloat16)

    in_maps = []
    for c in range(NCORES):
        b, g = divmod(c, 2)
        gsl = slice(DL * g, DL * (g + 1))
        m = {
            "xq_t": _bf16(query[b].T),
            "xk_t": _bf16(key_[b].T),
            "xv_t": _bf16(value[b].T),
            "wq_t": _bf16(Wq[gsl].T * 0.125),
            "wk_t": _bf16(Wk[gsl].T),
            "wv_t": _bf16(Wv[gsl].T),
            "wo_t": _bf16(Wo[:, gsl].T),
            "bq8": _bf16(bq[gsl].reshape(1, DL) * 0.125),
            "bk": _bf16(bk[gsl].reshape(1, DL)),
            "bv": _bf16(bv[gsl].reshape(1, DL)),
            "ones_row": ones_row,
        }
        if n_pat:
            m["maskp"] = mp
        in_maps.append(m)

    import os
    kwargs = {}
    if os.environ.get("BASS_MHA_TRACE"):
        _install_ntff_shim()
        tc_env = os.environ.get("BASS_MHA_TRACE_CORES", "0")
        cores = (list(range(NCORES)) if tc_env == "all"
                 else [int(x) for x in tc_env.split(",")])
        kwargs = dict(trace=True, trace_cores=cores)
    res = run_bass_kernel_spmd(nc, in_maps, core_ids=list(range(NCORES)), **kwargs)
    global LAST_RESULTS
    LAST_RESULTS = res

    out = np.empty((B, S, D), np.float32)
    for b in range(B):
        acc = res.results[2 * b]["outT"] + res.results[2 * b + 1]["outT"]
        out[b] = acc.T + bo[None, :]
    return out


# revision 33
# speedup vs baseline: 1.2530x; 1.0036x over previous
"""Multi-head attention (B=4, S=2048, D=1024, H=16) on 8 TRN2 NeuronCores.

Sharding: core c handles batch b=c//2 and head-group g=c%2 (8 heads, 512 of
the 1024 model dims).  Wq/Wk/Wv column-parallel, Wo row-parallel; the two
head-group partial outputs per batch are summed on the host (no collectives).

Per-core dataflow (bf16 matmuls, fp32 PSUM accumulate):
  phase 1: Q.T = (Wq/8) @ x.T   [512,2048]   per head-pair tiles [128,512]
           K.T = Wk @ x.T       [512,2048]
           V   = x @ Wv.T       stored head-interleaved with a ones column:
                                [128, 4, 8*65]
  phase 2 (per 512-wide q-block, per head-PAIR p):
           per k-tile unit: two row-tiled concurrent matmuls (K=64 each,
             array rows 0-63 / 64-127) -> scores.T for both heads into one
             [128,2,512] PSUM tile (2 banks)
           ONE exp on ACT over [128,2,w] (both heads)
           causal diagonal chunks masked in-place by gpsimd.affine_select
           per head: raw[65,512] += [V_h|1].T @ expT  (row 64 = denominator)
           normalize: DVE reciprocal_approx_fast on raw[64], gpsimd
             partition broadcast, DVE multiply -> attnT bf16
  phase 3 (per q-block): outT += Wo_g.T.T @ attnT_cat -> [1024,2048] partial
Host: out[b] = (partial_g0 + partial_g1).T + bo
"""

import numpy as np
import ml_dtypes
from contextlib import ExitStack

B = 4
S = 2048
D = 1024
H = 16
DK = 64
G = 2                 # head groups
HL = H // G           # heads per core = 8
DL = D // G           # local head dims = 512
QB = 512              # q-block width
CH = 128              # chunk / k-tile width
NKT = S // CH         # 16 k-tiles
NQB = S // QB         # 4 q-blocks
NCORES = 8
NPAIR = HL // 2       # 4 head pairs per core


def _bf16(x):
    return np.ascontiguousarray(x, dtype=np.float32).astype(ml_dtypes.bfloat16)


def _plan_from_mask(m):
    """m: [S, S] bool, True = masked (scores[q, k] masked).

    Returns (plans, patterns):
      plans[qb][kt] = None (skip) or (c0, nch, mixed) where mixed is a list of
        (rel_chunk, kind, val): kind 'affine' -> val = base offset for
        gpsimd.affine_select (valid iff q_global - k_global + 0 >= 0 with
        base = q0 - k0); kind 'pat' -> val = index into patterns.
      patterns: list of unique [128,128] float32 0/1 valid-masks (scoresT
        orientation: [k_partition, q_free]) for non-affine mixed chunks.
    """
    patterns = []
    pat_index = {}
    plans = []
    kk = np.arange(CH)
    for qb in range(NQB):
        row = []
        for kt in range(NKT):
            sub = m[qb * QB:(qb + 1) * QB, kt * CH:(kt + 1) * CH]  # [q, k]
            valid = (~sub).T  # [k, q] 128 x 512
            nchunks = QB // CH
            kinds = []
            for c in range(nchunks):
                ch = valid[:, c * CH:(c + 1) * CH]
                if ch.all():
                    kinds.append("full")
                elif not ch.any():
                    kinds.append("empty")
                else:
                    kinds.append("mixed")
            not_empty = [c for c in range(nchunks) if kinds[c] != "empty"]
            if not not_empty:
                row.append(None)
                continue
            c0, c1 = not_empty[0], not_empty[-1]
            mixed = []
            for c in range(c0, c1 + 1):
                if kinds[c] == "full":
                    continue
                pat = valid[:, c * CH:(c + 1) * CH]
                # affine (causal) check: valid[k, q] == (q0 + q >= k0 + k)
                q0 = qb * QB + c * CH
                k0 = kt * CH
                base = q0 - k0
                aff = (base + kk[None, :] - kk[:, None]) >= 0
                if (pat == aff).all():
                    mixed.append((c - c0, "affine", base))
                else:
                    key = pat.tobytes()
                    if key not in pat_index:
                        pat_index[key] = len(patterns)
                        patterns.append(pat.astype(np.float32))
                    mixed.append((c - c0, "pat", pat_index[key]))
            row.append((c0, c1 - c0 + 1, mixed))
        plans.append(row)
    return plans, patterns


def _build(plans, n_patterns, guard_empty_rows, has_bias):
    import concourse.bacc as bacc
    import concourse.tile as tile
    from concourse import mybir

    F32 = mybir.dt.float32
    BF16 = mybir.dt.bfloat16
    AF = mybir.ActivationFunctionType
    GE = mybir.AluOpType.is_ge

    nc = bacc.Bacc("TRN2", target_bir_lowering=False, debug=False)

    xq = nc.dram_tensor("xq_t", [D, S], BF16, kind="ExternalInput")
    xk = nc.dram_tensor("xk_t", [D, S], BF16, kind="ExternalInput")
    xv = nc.dram_tensor("xv_t", [D, S], BF16, kind="ExternalInput")
    wq = nc.dram_tensor("wq_t", [D, DL], BF16, kind="ExternalInput")
    wk = nc.dram_tensor("wk_t", [D, DL], BF16, kind="ExternalInput")
    wv = nc.dram_tensor("wv_t", [D, DL], BF16, kind="ExternalInput")
    wo = nc.dram_tensor("wo_t", [DL, D], BF16, kind="ExternalInput")
    bq = nc.dram_tensor("bq8", [1, DL], BF16, kind="ExternalInput")
    bk = nc.dram_tensor("bk", [1, DL], BF16, kind="ExternalInput")
    bv = nc.dram_tensor("bv", [1, DL], BF16, kind="ExternalInput")
    onesr = nc.dram_tensor("ones_row", [1, QB], BF16, kind="ExternalInput")
    if n_patterns:
        maskp = nc.dram_tensor("maskp", [CH, n_patterns * CH], BF16,
                               kind="ExternalInput")
    outT = nc.dram_tensor("outT", [D, S], F32, kind="ExternalOutput")
    import os as _os
    DBG = bool(_os.environ.get("BASS_MHA_DEBUG"))
    if DBG:
        qt_dbg = nc.dram_tensor("qt_dbg", [CH, QB], BF16, kind="ExternalOutput")
        kt_dbg = nc.dram_tensor("kt_dbg", [CH, QB], BF16, kind="ExternalOutput")
        vg_dbg = nc.dram_tensor("vg_dbg", [CH, HL * (DK + 1)], BF16, kind="ExternalOutput")
        ex_dbg = nc.dram_tensor("ex_dbg", [CH, 2 * QB], BF16, kind="ExternalOutput")
        att_dbg = nc.dram_tensor("att_dbg", [CH, (DL // CH) * QB], BF16, kind="ExternalOutput")

    MT = DL // CH      # 4 head-pair tiles
    NQU = S // QB      # 4 s-quarters
    NK = D // CH       # 8 contraction tiles

    with tile.TileContext(nc) as tc, ExitStack() as ctx:
        persist = ctx.enter_context(tc.tile_pool(name="persist", bufs=1))
        xin = ctx.enter_context(tc.tile_pool(name="xin", bufs=16))
        wt = ctx.enter_context(tc.tile_pool(name="wt", bufs=25))
        expp = ctx.enter_context(tc.tile_pool(name="expp", bufs=4))
        rawcp = ctx.enter_context(tc.tile_pool(name="rawcp", bufs=2))
        attp = ctx.enter_context(tc.tile_pool(name="attp", bufs=2))
        outp = ctx.enter_context(tc.tile_pool(name="outp", bufs=4))
        recp = ctx.enter_context(tc.tile_pool(name="recp", bufs=2))
        ps_mm = ctx.enter_context(tc.tile_pool(name="ps_mm", bufs=2, space="PSUM"))
        ps_sc = ctx.enter_context(tc.tile_pool(name="ps_sc", bufs=2, space="PSUM"))
        ps_raw = ctx.enter_context(tc.tile_pool(name="ps_raw", bufs=1, space="PSUM"))

        # per-(pair, quarter) projection output tiles: heads 2p / 2p+1 live on
        # partitions 0-63 / 64-127 -> row-tiled concurrent score matmuls
        qt_q = {(p, qu): persist.tile([CH, QB], BF16, name=f"qt_{p}_{qu}")
                for p in range(MT) for qu in range(NQU)}
        kt_q = {(p, qu): persist.tile([CH, QB], BF16, name=f"kt_{p}_{qu}")
                for p in range(MT) for qu in range(NQU)}
        v_g = [persist.tile([CH, NQU, HL * (DK + 1)], BF16, name=f"v_g{qu}")
               for qu in range(NQU)]
        wo_all = persist.tile([CH, MT, D], BF16)
        ones_sb = persist.tile([1, QB], BF16)
        bq_sb = persist.tile([1, DL], BF16)
        bk_sb = persist.tile([1, DL], BF16)
        bv_sb = persist.tile([1, DL], BF16)
        if n_patterns:
            mp_sb = persist.tile([CH, n_patterns, CH], BF16)

        nc.sync.dma_start(ones_sb[:], onesr.ap())
        nc.sync.dma_start(bq_sb[:], bq.ap())
        nc.sync.dma_start(bk_sb[:], bk.ap())
        nc.sync.dma_start(bv_sb[:], bv.ap())

        # ACT exp-table preload off the critical path + PE warm-up while the
        # first input DMAs land
        dum = persist.tile([1, 16], F32)
        wu_ps = ps_mm.tile([1, QB], F32, tag="mm")
        for _ in range(20):
            nc.tensor.matmul(wu_ps[:], ones_sb[0:1, 0:1], ones_sb[0:1, :],
                             start=True, stop=True)
        nc.scalar.activation(dum[:], wu_ps[0:1, 0:16], AF.Exp)

        def load_w(dram):
            tiles = []
            for kt in range(NK):
                wtile = wt.tile([CH, DL], BF16, tag="w")
                nc.sync.dma_start(wtile[:], dram.ap()[kt * CH:(kt + 1) * CH, :])
                tiles.append(wtile)
            return tiles

        def load_x(dram, qu):
            tiles = []
            for kt in range(NK):
                xt = xin.tile([CH, QB], BF16, tag="x")
                nc.sync.dma_start(
                    xt[:], dram.ap()[kt * CH:(kt + 1) * CH,
                                     qu * QB:(qu + 1) * QB])
                tiles.append(xt)
            return tiles

        def proj_qk_m(x_tiles, w_tiles, bias_sb, dst_map, qu, p, use_bias):
            ps = ps_mm.tile([CH, QB], F32, tag="mm")
            for kt in range(NK):
                nc.tensor.matmul(
                    ps[:], w_tiles[kt][:, p * CH:(p + 1) * CH],
                    x_tiles[kt][:], start=(kt == 0),
                    stop=(not use_bias and kt == NK - 1))
            if use_bias:
                nc.tensor.matmul(
                    ps[:], bias_sb[0:1, p * CH:(p + 1) * CH],
                    ones_sb[0:1, :], start=False, stop=True)
            nc.any.tensor_copy(out=dst_map[(p, qu)][:], in_=ps[:])

        def proj_v_j(x_tiles, qu, j):
            ps = ps_mm.tile([CH, DL], F32, tag="mm")
            for kt in range(NK):
                nc.tensor.matmul(
                    ps[:], x_tiles[kt][:, j * CH:(j + 1) * CH],
                    wv_t[kt][:], start=(kt == 0),
                    stop=(not has_bias[2] and kt == NK - 1))
            if has_bias[2]:
                nc.tensor.matmul(
                    ps[:], ones_sb[0:1, 0:CH], bv_sb[0:1, :],
                    start=False, stop=True)
            nc.vector.tensor_copy(
                out=v_g[qu][:, j, :].rearrange(
                    "p (h c) -> p h c", c=DK + 1)[:, :, 0:DK],
                in_=ps[:].rearrange("p (h c) -> p h c", c=DK),
            )

        zero_r = nc.gpsimd.to_reg(0.0)
        wq_t = []
        wv_t = []

        att_tiles = {}

        def attention_qb(qb):
            att = attp.tile([CH, MT, QB], BF16, tag="att")
            att_tiles[qb] = att
            units = [(kt,) + plans[qb][kt] for kt in range(NKT)
                     if plans[qb][kt] is not None]
            units = [(kt, c0, nch * CH, mixed)
                     for (kt, c0, nch, mixed) in units]
            for p in range(NPAIR):
                raw = ps_raw.tile([DK + 1, 2, QB], F32, tag="raw")
                nu = len(units)
                for ui, (kt, c0, w, mixed) in enumerate(units):
                    o = c0 * CH
                    sc = ps_sc.tile([CH, 2, QB], F32, tag="sc")
                    ktile = kt_q[(p, kt // 4)]
                    qtile = qt_q[(p, qb)]
                    ksl = slice((kt % 4) * CH, (kt % 4 + 1) * CH)
                    # two row-tiled concurrent matmuls (array rows 0-63/64-127)
                    nc.tensor.matmul(sc[:, 0, 0:w], ktile[0:DK, ksl],
                                     qtile[0:DK, o:o + w],
                                     start=True, stop=True)
                    nc.tensor.matmul(sc[:, 1, 0:w], ktile[DK:CH, ksl],
                                     qtile[DK:CH, o:o + w],
                                     start=True, stop=True)
                    ex = expp.tile([CH, 2, QB], BF16, tag="exp")
                    nc.scalar.activation(ex[:, :, 0:w], sc[:, :, 0:w], AF.Exp)
                    for (rel, kind, val) in mixed:
                        cs = slice(rel * CH, (rel + 1) * CH)
                        if kind == "affine":
                            nc.gpsimd.affine_select(
                                out=ex[:, :, cs], in_=ex[:, :, cs],
                                pattern=[[0, 2], [1, CH]],
                                compare_op=GE, fill=zero_r,
                                base=val, channel_multiplier=-1)
                        else:
                            for e in range(2):
                                nc.vector.tensor_mul(
                                    ex[:, e, cs], ex[:, e, cs],
                                    mp_sb[:, val, :])
                    if DBG and qb == 0 and p == 0 and kt == 0:
                        nc.sync.dma_start(
                            ex_dbg.ap().rearrange("p (a b) -> p a b", b=QB),
                            ex[:])
                    for e in range(2):
                        h = 2 * p + e
                        nc.tensor.matmul(
                            raw[:, e, o:o + w],
                            v_g[kt // 4][:, kt % 4,
                                         h * (DK + 1):(h + 1) * (DK + 1)],
                            ex[:, e, 0:w],
                            start=(ui == 0), stop=(ui == nu - 1))
                # single copy frees the raw PSUM banks for the next pair;
                # normalization then runs off the critical path from SBUF
                # (the custom-DVE reciprocal also requires SBUF input)
                rawc = rawcp.tile([DK + 1, 2, QB], F32, tag="rawc")
                nc.vector.tensor_copy(out=rawc[:], in_=raw[:])
                for e in range(2):
                    # stage the denominator at base partition 0: the custom-DVE
                    # reciprocal mishandles inputs at a nonzero base partition
                    rec = recp.tile([1, QB], F32, tag="rec")
                    den = recp.tile([1, QB], F32, tag="den")
                    nc.vector.tensor_scalar_max(
                        den[:], rawc[DK:DK + 1, e, :], 1e-30)
                    nc.vector.reciprocal_approx_fast(rec[:], den[:])
                    recb = recp.tile([DK, QB], F32, tag="recb")
                    nc.gpsimd.partition_broadcast(recb[:], rec[:])
                    nc.vector.tensor_mul(
                        att[e * DK:(e + 1) * DK, p, :], rawc[0:DK, e, :],
                        recb[:])

            if DBG and qb == 0:
                nc.sync.dma_start(
                    att_dbg.ap().rearrange("p (a b) -> p a b", b=QB), att[:])

        def outproj_qb(qb):
            att = att_tiles[qb]
            for mo in range(D // CH):
                ps = ps_mm.tile([CH, QB], F32, tag="mm")
                for ct in range(MT):
                    nc.tensor.matmul(
                        ps[:], wo_all[:, ct, mo * CH:(mo + 1) * CH],
                        att[:, ct, :], start=(ct == 0), stop=(ct == MT - 1))
                ot = outp.tile([CH, QB], F32, tag="ot")
                nc.any.tensor_copy(out=ot[:], in_=ps[:])
                nc.sync.dma_start(
                    outT.ap()[mo * CH:(mo + 1) * CH, qb * QB:(qb + 1) * QB],
                    ot[:])

        # ---- interleaved emission: proj rounds feed PE while attention ----
        # is exp(ACT)-paced; the scheduler fills PE bubbles with proj work.
        wk_t = load_w(wk)

        def proj_round(qu):
            xk_tiles = load_x(xk, qu)
            for i in range(MT):
                proj_qk_m(xk_tiles, wk_t, bk_sb, kt_q, qu, i, has_bias[1])
            if qu == 0:
                wq_t.extend(load_w(wq))
            xq_tiles = load_x(xq, qu)
            for i in range(MT):
                proj_qk_m(xq_tiles, wq_t, bq_sb, qt_q, qu, i, has_bias[0])
            if qu == 0:
                wv_t.extend(load_w(wv))
            xv_tiles = load_x(xv, qu)
            # ones column interleaves with V values at 65-element stride: all
            # v_g writers must be the SAME engine (DVE) — a DMA writing the
            # ones column races the V copies within shared SBUF lines.
            nc.vector.memset(
                v_g[qu][:].rearrange("p s (h c) -> p s h c",
                                     c=DK + 1)[:, :, :, DK:DK + 1], 1.0)
            for i in range(MT):
                proj_v_j(xv_tiles, qu, i)

        proj_round(0)
        if DBG:
            nc.sync.dma_start(qt_dbg.ap(), qt_q[(0, 0)][:])
            nc.sync.dma_start(kt_dbg.ap(), kt_q[(0, 0)][:])
            nc.sync.dma_start(vg_dbg.ap(), v_g[0][:, 0, :])
        # deferred bulk constants (needed from attention onward)
        if n_patterns:
            nc.sync.dma_start(mp_sb[:], maskp.ap().rearrange(
                "p (u f) -> p u f", f=CH))
        nc.sync.dma_start(wo_all[:], wo.ap().rearrange("(t p) m -> p t m", p=CH))
        attention_qb(0)
        proj_round(1)
        attention_qb(1)
        outproj_qb(0)
        proj_round(2)
        attention_qb(2)
        outproj_qb(1)
        proj_round(3)
        attention_qb(3)
        outproj_qb(2)
        outproj_qb(3)

    nc.compile()
    return nc


_CACHE = {}
LAST_RESULTS = None


def _install_ntff_shim():
    """Provide antenv.axon_hooks (NTFF profiling) when the image lacks it."""
    import sys, types, ctypes, contextlib
    if "antenv.axon_hooks" in sys.modules:
        return
    import antenv
    mod = types.ModuleType("antenv.axon_hooks")
    state = {"hook": None}
    mod.set_axon_ntff_profile_hook = lambda h: state.__setitem__("hook", h)
    mod.get_axon_ntff_profile_hook = lambda: state["hook"]
    sys.modules["antenv.axon_hooks"] = mod
    antenv.axon_hooks = mod
    try:
        lib = ctypes.CDLL("/opt/axon/libaxon_pjrt.so")
    except OSError:
        return
    if not hasattr(lib, "axon_start_nrt_profile"):
        return
    lib.axon_start_nrt_profile.argtypes = [
        ctypes.POINTER(ctypes.c_int64), ctypes.c_size_t]
    lib.axon_start_nrt_profile.restype = ctypes.c_int64
    lib.axon_stop_nrt_profile.argtypes = [ctypes.c_char_p]
    lib.axon_stop_nrt_profile.restype = ctypes.c_int64

    @contextlib.contextmanager
    def _hook(output_dir, device_ids):
        import jax
        jax.devices()
        if device_ids:
            ids = (ctypes.c_int64 * len(device_ids))(*device_ids)
            rc = lib.axon_start_nrt_profile(ids, len(device_ids))
        else:
            rc = lib.axon_start_nrt_profile(None, 0)
        if rc != 0:
            raise RuntimeError(f"axon_start_nrt_profile rc={rc}")
        try:
            yield
        finally:
            n = lib.axon_stop_nrt_profile(str(output_dir).encode())
            print(f"profile: {n} ntff file(s) in {output_dir}", file=sys.stderr)

    state["hook"] = _hook


def _get_nc(mask2d, has_bias):
    key = (hash(mask2d.tobytes()), has_bias)
    if key not in _CACHE:
        plans, patterns = _plan_from_mask(mask2d)
        # guard against fully-masked rows (reference maps softmax NaN -> 0)
        valid_any = (~mask2d).any(axis=1)
        guard = bool((~valid_any).any())
        _CACHE[key] = (_build(plans, len(patterns), guard, has_bias), patterns)
    return _CACHE[key]


def kernel(query, key, value, mask, Wq, bq, Wk, bk, Wv, bv, Wo, bo):
    from concourse.bass_utils import run_bass_kernel_spmd

    query = np.asarray(query, dtype=np.float32)
    key_ = np.asarray(key, dtype=np.float32)
    value = np.asarray(value, dtype=np.float32)
    mask2d = np.asarray(mask, dtype=bool).reshape(S, S)
    Wq = np.asarray(Wq, dtype=np.float32)
    Wk = np.asarray(Wk, dtype=np.float32)
    Wv = np.asarray(Wv, dtype=np.float32)
    Wo = np.asarray(Wo, dtype=np.float32)
    bq = np.asarray(bq, dtype=np.float32)
    bk = np.asarray(bk, dtype=np.float32)
    bv = np.asarray(bv, dtype=np.float32)
    bo = np.asarray(bo, dtype=np.float32)

    has_bias = (bool(bq.any()), bool(bk.any()), bool(bv.any()))
    nc, patterns = _get_nc(mask2d, has_bias)

    n_pat = len(patterns)
    if n_pat:
        mp = np.empty((CH, n_pat * CH), np.float32)
        for u, pat in enumerate(patterns):
            mp[:, u * CH:(u + 1) * CH] = pat
        mp = mp.astype(ml_dtypes.bfloat16)
    ones_row = np.ones((1, QB), ml_dtypes.bfloat16)

    in_maps = []
    for c in range(NCORES):
        b, g = divmod(c, 2)
        gsl = slice(DL * g, DL * (g + 1))
        m = {
            "xq_t": _bf16(query[b].T),
            "xk_t": _bf16(key_[b].T),
            "xv_t": _bf16(value[b].T),
            "wq_t": _bf16(Wq[gsl].T * 0.125),
            "wk_t": _bf16(Wk[gsl].T),
            "wv_t": _bf16(Wv[gsl].T),
            "wo_t": _bf16(Wo[:, gsl].T),
            "bq8": _bf16(bq[gsl].reshape(1, DL) * 0.125),
            "bk": _bf16(bk[gsl].reshape(1, DL)),
            "bv": _bf16(bv[gsl].reshape(1, DL)),
            "ones_row": ones_row,
        }
        if n_pat:
            m["maskp"] = mp
        in_maps.append(m)

    import os
    kwargs = {}
    if os.environ.get("BASS_MHA_TRACE"):
        _install_ntff_shim()
        tc_env = os.environ.get("BASS_MHA_TRACE_CORES", "0")
        cores = (list(range(NCORES)) if tc_env == "all"
                 else [int(x) for x in tc_env.split(",")])
        kwargs = dict(trace=True, trace_cores=cores)
    res = run_bass_kernel_spmd(nc, in_maps, core_ids=list(range(NCORES)), **kwargs)
    global LAST_RESULTS
    LAST_RESULTS = res

    out = np.empty((B, S, D), np.float32)
    for b in range(B):
        acc = res.results[2 * b]["outT"] + res.results[2 * b + 1]["outT"]
        out[b] = acc.T + bo[None, :]
    return out


# revision 34
# speedup vs baseline: 1.3148x; 1.0493x over previous
"""Multi-head attention (B=4, S=2048, D=1024, H=16) on 8 TRN2 NeuronCores.

Sharding: core c handles batch b=c//2 and head-group g=c%2 (8 heads, 512 of
the 1024 model dims).  Wq/Wk/Wv column-parallel, Wo row-parallel; the two
head-group partial outputs per batch are summed on the host (no collectives).

Per-core dataflow (bf16 matmuls, fp32 PSUM accumulate):
  phase 1: Q.T = (Wq/8) @ x.T   [512,2048]   per head-pair tiles [128,512]
           K.T = Wk @ x.T       [512,2048]
           V   = x @ Wv.T       stored head-interleaved with a ones column:
                                [128, 4, 8*65]
  phase 2 (per 512-wide q-block, per head-PAIR p):
           per k-tile unit: two row-tiled concurrent matmuls (K=64 each,
             array rows 0-63 / 64-127) -> scores.T for both heads into one
             [128,2,512] PSUM tile (2 banks)
           ONE exp on ACT over [128,2,w] (both heads)
           causal diagonal chunks masked in-place by gpsimd.affine_select
           per head: raw[65,512] += [V_h|1].T @ expT  (row 64 = denominator)
           normalize: DVE reciprocal_approx_fast on raw[64], gpsimd
             partition broadcast, DVE multiply -> attnT bf16
  phase 3 (per q-block): outT += Wo_g.T.T @ attnT_cat -> [1024,2048] partial
Host: out[b] = (partial_g0 + partial_g1).T + bo
"""

import numpy as np
import ml_dtypes
from contextlib import ExitStack

B = 4
S = 2048
D = 1024
H = 16
DK = 64
G = 2                 # head groups
HL = H // G           # heads per core = 8
DL = D // G           # local head dims = 512
QB = 512              # q-block width
CH = 128              # chunk / k-tile width
NKT = S // CH         # 16 k-tiles
NQB = S // QB         # 4 q-blocks
NCORES = 8
NPAIR = HL // 2       # 4 head pairs per core


def _bf16(x):
    return np.ascontiguousarray(x, dtype=np.float32).astype(ml_dtypes.bfloat16)


def _plan_from_mask(m):
    """m: [S, S] bool, True = masked (scores[q, k] masked).

    Returns (plans, patterns):
      plans[qb][kt] = None (skip) or (c0, nch, mixed) where mixed is a list of
        (rel_chunk, kind, val): kind 'affine' -> val = base offset for
        gpsimd.affine_select (valid iff q_global - k_global + 0 >= 0 with
        base = q0 - k0); kind 'pat' -> val = index into patterns.
      patterns: list of unique [128,128] float32 0/1 valid-masks (scoresT
        orientation: [k_partition, q_free]) for non-affine mixed chunks.
    """
    patterns = []
    pat_index = {}
    plans = []
    kk = np.arange(CH)
    for qb in range(NQB):
        row = []
        for kt in range(NKT):
            sub = m[qb * QB:(qb + 1) * QB, kt * CH:(kt + 1) * CH]  # [q, k]
            valid = (~sub).T  # [k, q] 128 x 512
            nchunks = QB // CH
            kinds = []
            for c in range(nchunks):
                ch = valid[:, c * CH:(c + 1) * CH]
                if ch.all():
                    kinds.append("full")
                elif not ch.any():
                    kinds.append("empty")
                else:
                    kinds.append("mixed")
            not_empty = [c for c in range(nchunks) if kinds[c] != "empty"]
            if not not_empty:
                row.append(None)
                continue
            c0, c1 = not_empty[0], not_empty[-1]
            mixed = []
            for c in range(c0, c1 + 1):
                if kinds[c] == "full":
                    continue
                pat = valid[:, c * CH:(c + 1) * CH]
                # affine (causal) check: valid[k, q] == (q0 + q >= k0 + k)
                q0 = qb * QB + c * CH
                k0 = kt * CH
                base = q0 - k0
                aff = (base + kk[None, :] - kk[:, None]) >= 0
                if (pat == aff).all():
                    mixed.append((c - c0, "affine", base))
                else:
                    key = pat.tobytes()
                    if key not in pat_index:
                        pat_index[key] = len(patterns)
                        patterns.append(pat.astype(np.float32))
                    mixed.append((c - c0, "pat", pat_index[key]))
            row.append((c0, c1 - c0 + 1, mixed))
        plans.append(row)
    return plans, patterns


def _build(plans, n_patterns, guard_empty_rows, has_bias):
    import concourse.bacc as bacc
    import concourse.tile as tile
    from concourse import mybir

    F32 = mybir.dt.float32
    BF16 = mybir.dt.bfloat16
    AF = mybir.ActivationFunctionType
    GE = mybir.AluOpType.is_ge

    nc = bacc.Bacc("TRN2", target_bir_lowering=False, debug=False)

    xq = nc.dram_tensor("xq_t", [D, S], BF16, kind="ExternalInput")
    xk = nc.dram_tensor("xk_t", [D, S], BF16, kind="ExternalInput")
    xv = nc.dram_tensor("xv_t", [D, S], BF16, kind="ExternalInput")
    wq = nc.dram_tensor("wq_t", [D, DL], BF16, kind="ExternalInput")
    wk = nc.dram_tensor("wk_t", [D, DL], BF16, kind="ExternalInput")
    wv = nc.dram_tensor("wv_t", [D, DL], BF16, kind="ExternalInput")
    wo = nc.dram_tensor("wo_t", [DL, D], BF16, kind="ExternalInput")
    bq = nc.dram_tensor("bq8", [1, DL], BF16, kind="ExternalInput")
    bk = nc.dram_tensor("bk", [1, DL], BF16, kind="ExternalInput")
    bv = nc.dram_tensor("bv", [1, DL], BF16, kind="ExternalInput")
    onesr = nc.dram_tensor("ones_row", [1, QB], BF16, kind="ExternalInput")
    if n_patterns:
        maskp = nc.dram_tensor("maskp", [CH, n_patterns * CH], BF16,
                               kind="ExternalInput")
    outT = nc.dram_tensor("outT", [D, S], F32, kind="ExternalOutput")
    import os as _os
    DBG = bool(_os.environ.get("BASS_MHA_DEBUG"))
    if DBG:
        qt_dbg = nc.dram_tensor("qt_dbg", [CH, QB], BF16, kind="ExternalOutput")
        kt_dbg = nc.dram_tensor("kt_dbg", [CH, QB], BF16, kind="ExternalOutput")
        vg_dbg = nc.dram_tensor("vg_dbg", [CH, HL * (DK + 1)], BF16, kind="ExternalOutput")
        ex_dbg = nc.dram_tensor("ex_dbg", [CH, 2 * QB], BF16, kind="ExternalOutput")
        att_dbg = nc.dram_tensor("att_dbg", [CH, (DL // CH) * QB], BF16, kind="ExternalOutput")

    MT = DL // CH      # 4 head-pair tiles
    NQU = S // QB      # 4 s-quarters
    NK = D // CH       # 8 contraction tiles

    with tile.TileContext(nc) as tc, ExitStack() as ctx:
        persist = ctx.enter_context(tc.tile_pool(name="persist", bufs=1))
        xin = ctx.enter_context(tc.tile_pool(name="xin", bufs=16))
        wt = ctx.enter_context(tc.tile_pool(name="wt", bufs=25))
        expp = ctx.enter_context(tc.tile_pool(name="expp", bufs=4))
        rawcp = ctx.enter_context(tc.tile_pool(name="rawcp", bufs=2))
        attp = ctx.enter_context(tc.tile_pool(name="attp", bufs=4))
        outp = ctx.enter_context(tc.tile_pool(name="outp", bufs=4))
        recp = ctx.enter_context(tc.tile_pool(name="recp", bufs=2))
        ps_mm = ctx.enter_context(tc.tile_pool(name="ps_mm", bufs=2, space="PSUM"))
        ps_sc = ctx.enter_context(tc.tile_pool(name="ps_sc", bufs=2, space="PSUM"))
        ps_raw = ctx.enter_context(tc.tile_pool(name="ps_raw", bufs=1, space="PSUM"))

        # per-(pair, quarter) projection output tiles: heads 2p / 2p+1 live on
        # partitions 0-63 / 64-127 -> row-tiled concurrent score matmuls
        qt_q = {(p, qu): persist.tile([CH, QB], BF16, name=f"qt_{p}_{qu}")
                for p in range(MT) for qu in range(NQU)}
        kt_q = {(p, qu): persist.tile([CH, QB], BF16, name=f"kt_{p}_{qu}")
                for p in range(MT) for qu in range(NQU)}
        v_g = [persist.tile([CH, NQU, HL * (DK + 1)], BF16, name=f"v_g{qu}")
               for qu in range(NQU)]
        wo_all = persist.tile([CH, MT, D], BF16)
        ones_sb = persist.tile([1, QB], BF16)
        bq_sb = persist.tile([1, DL], BF16)
        bk_sb = persist.tile([1, DL], BF16)
        bv_sb = persist.tile([1, DL], BF16)
        if n_patterns:
            mp_sb = persist.tile([CH, n_patterns, CH], BF16)

        nc.sync.dma_start(ones_sb[:], onesr.ap())
        nc.sync.dma_start(bq_sb[:], bq.ap())
        nc.sync.dma_start(bk_sb[:], bk.ap())
        nc.sync.dma_start(bv_sb[:], bv.ap())

        # ACT exp-table preload off the critical path + PE warm-up while the
        # first input DMAs land
        dum = persist.tile([1, 16], F32)
        wu_ps = ps_mm.tile([1, QB], F32, tag="mm")
        for _ in range(20):
            nc.tensor.matmul(wu_ps[:], ones_sb[0:1, 0:1], ones_sb[0:1, :],
                             start=True, stop=True)
        nc.scalar.activation(dum[:], wu_ps[0:1, 0:16], AF.Exp)

        def load_w(dram):
            tiles = []
            for kt in range(NK):
                wtile = wt.tile([CH, DL], BF16, tag="w")
                nc.sync.dma_start(wtile[:], dram.ap()[kt * CH:(kt + 1) * CH, :])
                tiles.append(wtile)
            return tiles

        def load_x(dram, qu):
            tiles = []
            for kt in range(NK):
                xt = xin.tile([CH, QB], BF16, tag="x")
                nc.sync.dma_start(
                    xt[:], dram.ap()[kt * CH:(kt + 1) * CH,
                                     qu * QB:(qu + 1) * QB])
                tiles.append(xt)
            return tiles

        def proj_qk_m(x_tiles, w_tiles, bias_sb, dst_map, qu, p, use_bias):
            ps = ps_mm.tile([CH, QB], F32, tag="mm")
            for kt in range(NK):
                nc.tensor.matmul(
                    ps[:], w_tiles[kt][:, p * CH:(p + 1) * CH],
                    x_tiles[kt][:], start=(kt == 0),
                    stop=(not use_bias and kt == NK - 1))
            if use_bias:
                nc.tensor.matmul(
                    ps[:], bias_sb[0:1, p * CH:(p + 1) * CH],
                    ones_sb[0:1, :], start=False, stop=True)
            nc.any.tensor_copy(out=dst_map[(p, qu)][:], in_=ps[:])

        def proj_v_j(x_tiles, qu, j):
            ps = ps_mm.tile([CH, DL], F32, tag="mm")
            for kt in range(NK):
                nc.tensor.matmul(
                    ps[:], x_tiles[kt][:, j * CH:(j + 1) * CH],
                    wv_t[kt][:], start=(kt == 0),
                    stop=(not has_bias[2] and kt == NK - 1))
            if has_bias[2]:
                nc.tensor.matmul(
                    ps[:], ones_sb[0:1, 0:CH], bv_sb[0:1, :],
                    start=False, stop=True)
            nc.vector.tensor_copy(
                out=v_g[qu][:, j, :].rearrange(
                    "p (h c) -> p h c", c=DK + 1)[:, :, 0:DK],
                in_=ps[:].rearrange("p (h c) -> p h c", c=DK),
            )

        zero_r = nc.gpsimd.to_reg(0.0)
        wq_t = []
        wv_t = []

        att_tiles = {}

        def attention_qb(qb):
            att = attp.tile([CH, MT, QB], BF16, tag="att")
            att_tiles[qb] = att
            units = [(kt,) + plans[qb][kt] for kt in range(NKT)
                     if plans[qb][kt] is not None]
            units = [(kt, c0, nch * CH, mixed)
                     for (kt, c0, nch, mixed) in units]
            for p in range(NPAIR):
                raw = ps_raw.tile([DK + 1, 2, QB], F32, tag="raw")
                nu = len(units)
                for ui, (kt, c0, w, mixed) in enumerate(units):
                    o = c0 * CH
                    sc = ps_sc.tile([CH, 2, QB], F32, tag="sc")
                    ktile = kt_q[(p, kt // 4)]
                    qtile = qt_q[(p, qb)]
                    ksl = slice((kt % 4) * CH, (kt % 4 + 1) * CH)
                    # two row-tiled concurrent matmuls (array rows 0-63/64-127)
                    nc.tensor.matmul(sc[:, 0, 0:w], ktile[0:DK, ksl],
                                     qtile[0:DK, o:o + w],
                                     start=True, stop=True)
                    nc.tensor.matmul(sc[:, 1, 0:w], ktile[DK:CH, ksl],
                                     qtile[DK:CH, o:o + w],
                                     start=True, stop=True)
                    ex = expp.tile([CH, 2, QB], BF16, tag="exp")
                    nc.scalar.activation(ex[:, :, 0:w], sc[:, :, 0:w], AF.Exp)
                    for (rel, kind, val) in mixed:
                        cs = slice(rel * CH, (rel + 1) * CH)
                        if kind == "affine":
                            nc.gpsimd.affine_select(
                                out=ex[:, :, cs], in_=ex[:, :, cs],
                                pattern=[[0, 2], [1, CH]],
                                compare_op=GE, fill=zero_r,
                                base=val, channel_multiplier=-1)
                        else:
                            for e in range(2):
                                nc.vector.tensor_mul(
                                    ex[:, e, cs], ex[:, e, cs],
                                    mp_sb[:, val, :])
                    if DBG and qb == 0 and p == 0 and kt == 0:
                        nc.sync.dma_start(
                            ex_dbg.ap().rearrange("p (a b) -> p a b", b=QB),
                            ex[:])
                    for e in range(2):
                        h = 2 * p + e
                        nc.tensor.matmul(
                            raw[:, e, o:o + w],
                            v_g[kt // 4][:, kt % 4,
                                         h * (DK + 1):(h + 1) * (DK + 1)],
                            ex[:, e, 0:w],
                            start=(ui == 0), stop=(ui == nu - 1))
                # single copy frees the raw PSUM banks for the next pair;
                # normalization then runs off the critical path from SBUF
                # (the custom-DVE reciprocal also requires SBUF input)
                rawc = rawcp.tile([DK + 1, 2, QB], F32, tag="rawc")
                nc.vector.tensor_copy(out=rawc[:], in_=raw[:])
                for e in range(2):
                    # stage the denominator at base partition 0: the custom-DVE
                    # reciprocal mishandles inputs at a nonzero base partition
                    rec = recp.tile([1, QB], F32, tag="rec")
                    den = recp.tile([1, QB], F32, tag="den")
                    nc.vector.tensor_scalar_max(
                        den[:], rawc[DK:DK + 1, e, :], 1e-30)
                    nc.vector.reciprocal_approx_fast(rec[:], den[:])
                    recb = recp.tile([DK, QB], F32, tag="recb")
                    nc.gpsimd.partition_broadcast(recb[:], rec[:])
                    nc.vector.tensor_mul(
                        att[e * DK:(e + 1) * DK, p, :], rawc[0:DK, e, :],
                        recb[:])

            if DBG and qb == 0:
                nc.sync.dma_start(
                    att_dbg.ap().rearrange("p (a b) -> p a b", b=QB), att[:])

        def outproj_qb(qb):
            att = att_tiles[qb]
            for mo in range(D // CH):
                ps = ps_mm.tile([CH, QB], F32, tag="mm")
                for ct in range(MT):
                    nc.tensor.matmul(
                        ps[:], wo_all[:, ct, mo * CH:(mo + 1) * CH],
                        att[:, ct, :], start=(ct == 0), stop=(ct == MT - 1))
                ot = outp.tile([CH, QB], F32, tag="ot")
                nc.any.tensor_copy(out=ot[:], in_=ps[:])
                nc.sync.dma_start(
                    outT.ap()[mo * CH:(mo + 1) * CH, qb * QB:(qb + 1) * QB],
                    ot[:])

        # ---- interleaved emission: proj rounds feed PE while attention ----
        # is exp(ACT)-paced; the scheduler fills PE bubbles with proj work.
        wk_t = load_w(wk)

        def proj_round(qu):
            xk_tiles = load_x(xk, qu)
            for i in range(MT):
                proj_qk_m(xk_tiles, wk_t, bk_sb, kt_q, qu, i, has_bias[1])
            if qu == 0:
                wq_t.extend(load_w(wq))
            xq_tiles = load_x(xq, qu)
            for i in range(MT):
                proj_qk_m(xq_tiles, wq_t, bq_sb, qt_q, qu, i, has_bias[0])
            if qu == 0:
                wv_t.extend(load_w(wv))
            xv_tiles = load_x(xv, qu)
            # ones column interleaves with V values at 65-element stride: all
            # v_g writers must be the SAME engine (DVE) — a DMA writing the
            # ones column races the V copies within shared SBUF lines.
            nc.vector.memset(
                v_g[qu][:].rearrange("p s (h c) -> p s h c",
                                     c=DK + 1)[:, :, :, DK:DK + 1], 1.0)
            for i in range(MT):
                proj_v_j(xv_tiles, qu, i)

        proj_round(0)
        if DBG:
            nc.sync.dma_start(qt_dbg.ap(), qt_q[(0, 0)][:])
            nc.sync.dma_start(kt_dbg.ap(), kt_q[(0, 0)][:])
            nc.sync.dma_start(vg_dbg.ap(), v_g[0][:, 0, :])
        # deferred bulk constants (needed from attention onward)
        if n_patterns:
            nc.sync.dma_start(mp_sb[:], maskp.ap().rearrange(
                "p (u f) -> p u f", f=CH))
        nc.sync.dma_start(wo_all[:], wo.ap().rearrange("(t p) m -> p t m", p=CH))
        attention_qb(0)
        proj_round(1)
        attention_qb(1)
        proj_round(2)
        attention_qb(2)
        proj_round(3)
        attention_qb(3)
        outproj_qb(0)
        outproj_qb(1)
        outproj_qb(2)
        outproj_qb(3)

    nc.compile()
    return nc


_CACHE = {}
LAST_RESULTS = None


def _install_ntff_shim():
    """Provide antenv.axon_hooks (NTFF profiling) when the image lacks it."""
    import sys, types, ctypes, contextlib
    if "antenv.axon_hooks" in sys.modules:
        return
    import antenv
    mod = types.ModuleType("antenv.axon_hooks")
    state = {"hook": None}
    mod.set_axon_ntff_profile_hook = lambda h: state.__setitem__("hook", h)
    mod.get_axon_ntff_profile_hook = lambda: state["hook"]
    sys.modules["antenv.axon_hooks"] = mod
    antenv.axon_hooks = mod
    try:
        lib = ctypes.CDLL("/opt/axon/libaxon_pjrt.so")
    except OSError:
        return
    if not hasattr(lib, "axon_start_nrt_profile"):
        return
    lib.axon_start_nrt_profile.argtypes = [
        ctypes.POINTER(ctypes.c_int64), ctypes.c_size_t]
    lib.axon_start_nrt_profile.restype = ctypes.c_int64
    lib.axon_stop_nrt_profile.argtypes = [ctypes.c_char_p]
    lib.axon_stop_nrt_profile.restype = ctypes.c_int64

    @contextlib.contextmanager
    def _hook(output_dir, device_ids):
        import jax
        jax.devices()
        if device_ids:
            ids = (ctypes.c_int64 * len(device_ids))(*device_ids)
            rc = lib.axon_start_nrt_profile(ids, len(device_ids))
        else:
            rc = lib.axon_start_nrt_profile(None, 0)
        if rc != 0:
            raise RuntimeError(f"axon_start_nrt_profile rc={rc}")
        try:
            yield
        finally:
            n = lib.axon_stop_nrt_profile(str(output_dir).encode())
            print(f"profile: {n} ntff file(s) in {output_dir}", file=sys.stderr)

    state["hook"] = _hook


def _get_nc(mask2d, has_bias):
    key = (hash(mask2d.tobytes()), has_bias)
    if key not in _CACHE:
        plans, patterns = _plan_from_mask(mask2d)
        # guard against fully-masked rows (reference maps softmax NaN -> 0)
        valid_any = (~mask2d).any(axis=1)
        guard = bool((~valid_any).any())
        _CACHE[key] = (_build(plans, len(patterns), guard, has_bias), patterns)
    return _CACHE[key]


def kernel(query, key, value, mask, Wq, bq, Wk, bk, Wv, bv, Wo, bo):
    from concourse.bass_utils import run_bass_kernel_spmd

    query = np.asarray(query, dtype=np.float32)
    key_ = np.asarray(key, dtype=np.float32)
    value = np.asarray(value, dtype=np.float32)
    mask2d = np.asarray(mask, dtype=bool).reshape(S, S)
    Wq = np.asarray(Wq, dtype=np.float32)
    Wk = np.asarray(Wk, dtype=np.float32)
    Wv = np.asarray(Wv, dtype=np.float32)
    Wo = np.asarray(Wo, dtype=np.float32)
    bq = np.asarray(bq, dtype=np.float32)
    bk = np.asarray(bk, dtype=np.float32)
    bv = np.asarray(bv, dtype=np.float32)
    bo = np.asarray(bo, dtype=np.float32)

    has_bias = (bool(bq.any()), bool(bk.any()), bool(bv.any()))
    nc, patterns = _get_nc(mask2d, has_bias)

    n_pat = len(patterns)
    if n_pat:
        mp = np.empty((CH, n_pat * CH), np.float32)
        for u, pat in enumerate(patterns):
            mp[:, u * CH:(u + 1) * CH] = pat
        mp = mp.astype(ml_dtypes.bfloat16)
    ones_row = np.ones((1, QB), ml_dtypes.bfloat16)

    in_maps = []
    for c in range(NCORES):
        b, g = divmod(c, 2)
        gsl = slice(DL * g, DL * (g + 1))
        m = {
            "xq_t": _bf16(query[b].T),
            "xk_t": _bf16(key_[b].T),
            "xv_t": _bf16(value[b].T),
            "wq_t": _bf16(Wq[gsl].T * 0.125),
            "wk_t": _bf16(Wk[gsl].T),
            "wv_t": _bf16(Wv[gsl].T),
            "wo_t": _bf16(Wo[:, gsl].T),
            "bq8": _bf16(bq[gsl].reshape(1, DL) * 0.125),
            "bk": _bf16(bk[gsl].reshape(1, DL)),
            "bv": _bf16(bv[gsl].reshape(1, DL)),
            "ones_row": ones_row,
        }
        if n_pat:
            m["maskp"] = mp
        in_maps.append(m)

    import os
    kwargs = {}
    if os.environ.get("BASS_MHA_TRACE"):
        _install_ntff_shim()
        tc_env = os.environ.get("BASS_MHA_TRACE_CORES", "0")
        cores = (list(range(NCORES)) if tc_env == "all"
                 else [int(x) for x in tc_env.split(",")])
        kwargs = dict(trace=True, trace_cores=cores)
    res = run_bass_kernel_spmd(nc, in_maps, core_ids=list(range(NCORES)), **kwargs)
    global LAST_RESULTS
    LAST_RESULTS = res

    out = np.empty((B, S, D), np.float32)
    for b in range(B):
        acc = res.results[2 * b]["outT"] + res.results[2 * b + 1]["outT"]
        out[b] = acc.T + bo[None, :]
    return out


# revision 35
# speedup vs baseline: 1.3183x; 1.0027x over previous
"""Multi-head attention (B=4, S=2048, D=1024, H=16) on 8 TRN2 NeuronCores.

Sharding: core c handles batch b=c//2 and head-group g=c%2 (8 heads, 512 of
the 1024 model dims).  Wq/Wk/Wv column-parallel, Wo row-parallel; the two
head-group partial outputs per batch are summed on the host (no collectives).

Per-core dataflow (bf16 matmuls, fp32 PSUM accumulate):
  phase 1: Q.T = (Wq/8) @ x.T   [512,2048]   per head-pair tiles [128,512]
           K.T = Wk @ x.T       [512,2048]
           V   = x @ Wv.T       stored head-interleaved with a ones column:
                                [128, 4, 8*65]
  phase 2 (per 512-wide q-block, per head-PAIR p):
           per k-tile unit: two row-tiled concurrent matmuls (K=64 each,
             array rows 0-63 / 64-127) -> scores.T for both heads into one
             [128,2,512] PSUM tile (2 banks)
           ONE exp on ACT over [128,2,w] (both heads)
           causal diagonal chunks masked in-place by gpsimd.affine_select
           per head: raw[65,512] += [V_h|1].T @ expT  (row 64 = denominator)
           normalize: DVE reciprocal_approx_fast on raw[64], gpsimd
             partition broadcast, DVE multiply -> attnT bf16
  phase 3 (per q-block): outT += Wo_g.T.T @ attnT_cat -> [1024,2048] partial
Host: out[b] = (partial_g0 + partial_g1).T + bo
"""

import numpy as np
import ml_dtypes
from contextlib import ExitStack

B = 4
S = 2048
D = 1024
H = 16
DK = 64
G = 2                 # head groups
HL = H // G           # heads per core = 8
DL = D // G           # local head dims = 512
QB = 512              # q-block width
CH = 128              # chunk / k-tile width
NKT = S // CH         # 16 k-tiles
NQB = S // QB         # 4 q-blocks
NCORES = 8
NPAIR = HL // 2       # 4 head pairs per core


def _bf16(x):
    return np.ascontiguousarray(x, dtype=np.float32).astype(ml_dtypes.bfloat16)


def _plan_from_mask(m):
    """m: [S, S] bool, True = masked (scores[q, k] masked).

    Returns (plans, patterns):
      plans[qb][kt] = None (skip) or (c0, nch, mixed) where mixed is a list of
        (rel_chunk, kind, val): kind 'affine' -> val = base offset for
        gpsimd.affine_select (valid iff q_global - k_global + 0 >= 0 with
        base = q0 - k0); kind 'pat' -> val = index into patterns.
      patterns: list of unique [128,128] float32 0/1 valid-masks (scoresT
        orientation: [k_partition, q_free]) for non-affine mixed chunks.
    """
    patterns = []
    pat_index = {}
    plans = []
    kk = np.arange(CH)
    for qb in range(NQB):
        row = []
        for kt in range(NKT):
            sub = m[qb * QB:(qb + 1) * QB, kt * CH:(kt + 1) * CH]  # [q, k]
            valid = (~sub).T  # [k, q] 128 x 512
            nchunks = QB // CH
            kinds = []
            for c in range(nchunks):
                ch = valid[:, c * CH:(c + 1) * CH]
                if ch.all():
                    kinds.append("full")
                elif not ch.any():
                    kinds.append("empty")
                else:
                    kinds.append("mixed")
            not_empty = [c for c in range(nchunks) if kinds[c] != "empty"]
            if not not_empty:
                row.append(None)
                continue
            c0, c1 = not_empty[0], not_empty[-1]
            mixed = []
            for c in range(c0, c1 + 1):
                if kinds[c] == "full":
                    continue
                pat = valid[:, c * CH:(c + 1) * CH]
                # affine (causal) check: valid[k, q] == (q0 + q >= k0 + k)
                q0 = qb * QB + c * CH
                k0 = kt * CH
                base = q0 - k0
                aff = (base + kk[None, :] - kk[:, None]) >= 0
                if (pat == aff).all():
                    mixed.append((c - c0, "affine", base))
                else:
                    key = pat.tobytes()
                    if key not in pat_index:
                        pat_index[key] = len(patterns)
                        patterns.append(pat.astype(np.float32))
                    mixed.append((c - c0, "pat", pat_index[key]))
            row.append((c0, c1 - c0 + 1, mixed))
        plans.append(row)
    return plans, patterns


def _build(plans, n_patterns, guard_empty_rows, has_bias):
    import concourse.bacc as bacc
    import concourse.tile as tile
    from concourse import mybir

    F32 = mybir.dt.float32
    BF16 = mybir.dt.bfloat16
    AF = mybir.ActivationFunctionType
    GE = mybir.AluOpType.is_ge

    nc = bacc.Bacc("TRN2", target_bir_lowering=False, debug=False)

    xq = nc.dram_tensor("xq_t", [D, S], BF16, kind="ExternalInput")
    xk = nc.dram_tensor("xk_t", [D, S], BF16, kind="ExternalInput")
    xv = nc.dram_tensor("xv_t", [D, S], BF16, kind="ExternalInput")
    wq = nc.dram_tensor("wq_t", [D, DL], BF16, kind="ExternalInput")
    wk = nc.dram_tensor("wk_t", [D, DL], BF16, kind="ExternalInput")
    wv = nc.dram_tensor("wv_t", [D, DL], BF16, kind="ExternalInput")
    wo = nc.dram_tensor("wo_t", [DL, D], BF16, kind="ExternalInput")
    bq = nc.dram_tensor("bq8", [1, DL], BF16, kind="ExternalInput")
    bk = nc.dram_tensor("bk", [1, DL], BF16, kind="ExternalInput")
    bv = nc.dram_tensor("bv", [1, DL], BF16, kind="ExternalInput")
    onesr = nc.dram_tensor("ones_row", [1, QB], BF16, kind="ExternalInput")
    if n_patterns:
        maskp = nc.dram_tensor("maskp", [CH, n_patterns * CH], BF16,
                               kind="ExternalInput")
    outT = nc.dram_tensor("outT", [D, S], F32, kind="ExternalOutput")
    import os as _os
    DBG = bool(_os.environ.get("BASS_MHA_DEBUG"))
    if DBG:
        qt_dbg = nc.dram_tensor("qt_dbg", [CH, QB], BF16, kind="ExternalOutput")
        kt_dbg = nc.dram_tensor("kt_dbg", [CH, QB], BF16, kind="ExternalOutput")
        vg_dbg = nc.dram_tensor("vg_dbg", [CH, HL * (DK + 1)], BF16, kind="ExternalOutput")
        ex_dbg = nc.dram_tensor("ex_dbg", [CH, 2 * QB], BF16, kind="ExternalOutput")
        att_dbg = nc.dram_tensor("att_dbg", [CH, (DL // CH) * QB], BF16, kind="ExternalOutput")

    MT = DL // CH      # 4 head-pair tiles
    NQU = S // QB      # 4 s-quarters
    NK = D // CH       # 8 contraction tiles

    with tile.TileContext(nc) as tc, ExitStack() as ctx:
        persist = ctx.enter_context(tc.tile_pool(name="persist", bufs=1))
        xin = ctx.enter_context(tc.tile_pool(name="xin", bufs=32))
        wt = ctx.enter_context(tc.tile_pool(name="wt", bufs=25))
        expp = ctx.enter_context(tc.tile_pool(name="expp", bufs=4))
        rawcp = ctx.enter_context(tc.tile_pool(name="rawcp", bufs=2))
        attp = ctx.enter_context(tc.tile_pool(name="attp", bufs=4))
        outp = ctx.enter_context(tc.tile_pool(name="outp", bufs=4))
        recp = ctx.enter_context(tc.tile_pool(name="recp", bufs=2))
        ps_mm = ctx.enter_context(tc.tile_pool(name="ps_mm", bufs=2, space="PSUM"))
        ps_sc = ctx.enter_context(tc.tile_pool(name="ps_sc", bufs=2, space="PSUM"))
        ps_raw = ctx.enter_context(tc.tile_pool(name="ps_raw", bufs=1, space="PSUM"))

        # per-(pair, quarter) projection output tiles: heads 2p / 2p+1 live on
        # partitions 0-63 / 64-127 -> row-tiled concurrent score matmuls
        qt_q = {(p, qu): persist.tile([CH, QB], BF16, name=f"qt_{p}_{qu}")
                for p in range(MT) for qu in range(NQU)}
        kt_q = {(p, qu): persist.tile([CH, QB], BF16, name=f"kt_{p}_{qu}")
                for p in range(MT) for qu in range(NQU)}
        v_g = [persist.tile([CH, NQU, HL * (DK + 1)], BF16, name=f"v_g{qu}")
               for qu in range(NQU)]
        wo_all = persist.tile([CH, MT, D], BF16)
        ones_sb = persist.tile([1, QB], BF16)
        bq_sb = persist.tile([1, DL], BF16)
        bk_sb = persist.tile([1, DL], BF16)
        bv_sb = persist.tile([1, DL], BF16)
        if n_patterns:
            mp_sb = persist.tile([CH, n_patterns, CH], BF16)

        nc.sync.dma_start(ones_sb[:], onesr.ap())
        nc.sync.dma_start(bq_sb[:], bq.ap())
        nc.sync.dma_start(bk_sb[:], bk.ap())
        nc.sync.dma_start(bv_sb[:], bv.ap())

        # ACT exp-table preload off the critical path + PE warm-up while the
        # first input DMAs land
        dum = persist.tile([1, 16], F32)
        wu_ps = ps_mm.tile([1, QB], F32, tag="mm")
        for _ in range(20):
            nc.tensor.matmul(wu_ps[:], ones_sb[0:1, 0:1], ones_sb[0:1, :],
                             start=True, stop=True)
        nc.scalar.activation(dum[:], wu_ps[0:1, 0:16], AF.Exp)

        def load_w(dram):
            tiles = []
            for kt in range(NK):
                wtile = wt.tile([CH, DL], BF16, tag="w")
                nc.sync.dma_start(wtile[:], dram.ap()[kt * CH:(kt + 1) * CH, :])
                tiles.append(wtile)
            return tiles

        def load_x(dram, qu):
            tiles = []
            for kt in range(NK):
                xt = xin.tile([CH, QB], BF16, tag="x")
                nc.sync.dma_start(
                    xt[:], dram.ap()[kt * CH:(kt + 1) * CH,
                                     qu * QB:(qu + 1) * QB])
                tiles.append(xt)
            return tiles

        def proj_qk_m(x_tiles, w_tiles, bias_sb, dst_map, qu, p, use_bias):
            ps = ps_mm.tile([CH, QB], F32, tag="mm")
            for kt in range(NK):
                nc.tensor.matmul(
                    ps[:], w_tiles[kt][:, p * CH:(p + 1) * CH],
                    x_tiles[kt][:], start=(kt == 0),
                    stop=(not use_bias and kt == NK - 1))
            if use_bias:
                nc.tensor.matmul(
                    ps[:], bias_sb[0:1, p * CH:(p + 1) * CH],
                    ones_sb[0:1, :], start=False, stop=True)
            nc.any.tensor_copy(out=dst_map[(p, qu)][:], in_=ps[:])

        def proj_v_j(x_tiles, qu, j):
            ps = ps_mm.tile([CH, DL], F32, tag="mm")
            for kt in range(NK):
                nc.tensor.matmul(
                    ps[:], x_tiles[kt][:, j * CH:(j + 1) * CH],
                    wv_t[kt][:], start=(kt == 0),
                    stop=(not has_bias[2] and kt == NK - 1))
            if has_bias[2]:
                nc.tensor.matmul(
                    ps[:], ones_sb[0:1, 0:CH], bv_sb[0:1, :],
                    start=False, stop=True)
            nc.vector.tensor_copy(
                out=v_g[qu][:, j, :].rearrange(
                    "p (h c) -> p h c", c=DK + 1)[:, :, 0:DK],
                in_=ps[:].rearrange("p (h c) -> p h c", c=DK),
            )

        zero_r = nc.gpsimd.to_reg(0.0)
        wq_t = []
        wv_t = []

        att_tiles = {}

        def attention_qb(qb):
            att = attp.tile([CH, MT, QB], BF16, tag="att")
            att_tiles[qb] = att
            units = [(kt,) + plans[qb][kt] for kt in range(NKT)
                     if plans[qb][kt] is not None]
            units = [(kt, c0, nch * CH, mixed)
                     for (kt, c0, nch, mixed) in units]
            for p in range(NPAIR):
                raw = ps_raw.tile([DK + 1, 2, QB], F32, tag="raw")
                nu = len(units)
                for ui, (kt, c0, w, mixed) in enumerate(units):
                    o = c0 * CH
                    sc = ps_sc.tile([CH, 2, QB], F32, tag="sc")
                    ktile = kt_q[(p, kt // 4)]
                    qtile = qt_q[(p, qb)]
                    ksl = slice((kt % 4) * CH, (kt % 4 + 1) * CH)
                    # two row-tiled concurrent matmuls (array rows 0-63/64-127)
                    nc.tensor.matmul(sc[:, 0, 0:w], ktile[0:DK, ksl],
                                     qtile[0:DK, o:o + w],
                                     start=True, stop=True)
                    nc.tensor.matmul(sc[:, 1, 0:w], ktile[DK:CH, ksl],
                                     qtile[DK:CH, o:o + w],
                                     start=True, stop=True)
                    ex = expp.tile([CH, 2, QB], BF16, tag="exp")
                    nc.scalar.activation(ex[:, :, 0:w], sc[:, :, 0:w], AF.Exp)
                    for (rel, kind, val) in mixed:
                        cs = slice(rel * CH, (rel + 1) * CH)
                        if kind == "affine":
                            nc.gpsimd.affine_select(
                                out=ex[:, :, cs], in_=ex[:, :, cs],
                                pattern=[[0, 2], [1, CH]],
                                compare_op=GE, fill=zero_r,
                                base=val, channel_multiplier=-1)
                        else:
                            for e in range(2):
                                nc.vector.tensor_mul(
                                    ex[:, e, cs], ex[:, e, cs],
                                    mp_sb[:, val, :])
                    if DBG and qb == 0 and p == 0 and kt == 0:
                        nc.sync.dma_start(
                            ex_dbg.ap().rearrange("p (a b) -> p a b", b=QB),
                            ex[:])
                    for e in range(2):
                        h = 2 * p + e
                        nc.tensor.matmul(
                            raw[:, e, o:o + w],
                            v_g[kt // 4][:, kt % 4,
                                         h * (DK + 1):(h + 1) * (DK + 1)],
                            ex[:, e, 0:w],
                            start=(ui == 0), stop=(ui == nu - 1))
                # single copy frees the raw PSUM banks for the next pair;
                # normalization then runs off the critical path from SBUF
                # (the custom-DVE reciprocal also requires SBUF input)
                rawc = rawcp.tile([DK + 1, 2, QB], F32, tag="rawc")
                nc.vector.tensor_copy(out=rawc[:], in_=raw[:])
                for e in range(2):
                    # stage the denominator at base partition 0: the custom-DVE
                    # reciprocal mishandles inputs at a nonzero base partition
                    rec = recp.tile([1, QB], F32, tag="rec")
                    den = recp.tile([1, QB], F32, tag="den")
                    nc.vector.tensor_scalar_max(
                        den[:], rawc[DK:DK + 1, e, :], 1e-30)
                    nc.vector.reciprocal_approx_fast(rec[:], den[:])
                    recb = recp.tile([DK, QB], F32, tag="recb")
                    nc.gpsimd.partition_broadcast(recb[:], rec[:])
                    nc.vector.tensor_mul(
                        att[e * DK:(e + 1) * DK, p, :], rawc[0:DK, e, :],
                        recb[:])

            if DBG and qb == 0:
                nc.sync.dma_start(
                    att_dbg.ap().rearrange("p (a b) -> p a b", b=QB), att[:])

        def outproj_qb(qb):
            att = att_tiles[qb]
            for mo in range(D // CH):
                ps = ps_mm.tile([CH, QB], F32, tag="mm")
                for ct in range(MT):
                    nc.tensor.matmul(
                        ps[:], wo_all[:, ct, mo * CH:(mo + 1) * CH],
                        att[:, ct, :], start=(ct == 0), stop=(ct == MT - 1))
                ot = outp.tile([CH, QB], F32, tag="ot")
                nc.any.tensor_copy(out=ot[:], in_=ps[:])
                nc.sync.dma_start(
                    outT.ap()[mo * CH:(mo + 1) * CH, qb * QB:(qb + 1) * QB],
                    ot[:])

        # ---- interleaved emission: proj rounds feed PE while attention ----
        # is exp(ACT)-paced; the scheduler fills PE bubbles with proj work.
        wk_t = load_w(wk)

        def proj_round(qu):
            xk_tiles = load_x(xk, qu)
            for i in range(MT):
                proj_qk_m(xk_tiles, wk_t, bk_sb, kt_q, qu, i, has_bias[1])
            if qu == 0:
                wq_t.extend(load_w(wq))
            xq_tiles = load_x(xq, qu)
            for i in range(MT):
                proj_qk_m(xq_tiles, wq_t, bq_sb, qt_q, qu, i, has_bias[0])
            if qu == 0:
                wv_t.extend(load_w(wv))
            xv_tiles = load_x(xv, qu)
            # ones column interleaves with V values at 65-element stride: all
            # v_g writers must be the SAME engine (DVE) — a DMA writing the
            # ones column races the V copies within shared SBUF lines.
            nc.vector.memset(
                v_g[qu][:].rearrange("p s (h c) -> p s h c",
                                     c=DK + 1)[:, :, :, DK:DK + 1], 1.0)
            for i in range(MT):
                proj_v_j(xv_tiles, qu, i)

        proj_round(0)
        if DBG:
            nc.sync.dma_start(qt_dbg.ap(), qt_q[(0, 0)][:])
            nc.sync.dma_start(kt_dbg.ap(), kt_q[(0, 0)][:])
            nc.sync.dma_start(vg_dbg.ap(), v_g[0][:, 0, :])
        # deferred bulk constants (needed from attention onward)
        if n_patterns:
            nc.sync.dma_start(mp_sb[:], maskp.ap().rearrange(
                "p (u f) -> p u f", f=CH))
        nc.sync.dma_start(wo_all[:], wo.ap().rearrange("(t p) m -> p t m", p=CH))
        attention_qb(0)
        proj_round(1)
        attention_qb(1)
        proj_round(2)
        attention_qb(2)
        proj_round(3)
        attention_qb(3)
        outproj_qb(0)
        outproj_qb(1)
        outproj_qb(2)
        outproj_qb(3)

    nc.compile()
    return nc


_CACHE = {}
LAST_RESULTS = None


def _install_ntff_shim():
    """Provide antenv.axon_hooks (NTFF profiling) when the image lacks it."""
    import sys, types, ctypes, contextlib
    if "antenv.axon_hooks" in sys.modules:
        return
    import antenv
    mod = types.ModuleType("antenv.axon_hooks")
    state = {"hook": None}
    mod.set_axon_ntff_profile_hook = lambda h: state.__setitem__("hook", h)
    mod.get_axon_ntff_profile_hook = lambda: state["hook"]
    sys.modules["antenv.axon_hooks"] = mod
    antenv.axon_hooks = mod
    try:
        lib = ctypes.CDLL("/opt/axon/libaxon_pjrt.so")
    except OSError:
        return
    if not hasattr(lib, "axon_start_nrt_profile"):
        return
    lib.axon_start_nrt_profile.argtypes = [
        ctypes.POINTER(ctypes.c_int64), ctypes.c_size_t]
    lib.axon_start_nrt_profile.restype = ctypes.c_int64
    lib.axon_stop_nrt_profile.argtypes = [ctypes.c_char_p]
    lib.axon_stop_nrt_profile.restype = ctypes.c_int64

    @contextlib.contextmanager
    def _hook(output_dir, device_ids):
        import jax
        jax.devices()
        if device_ids:
            ids = (ctypes.c_int64 * len(device_ids))(*device_ids)
            rc = lib.axon_start_nrt_profile(ids, len(device_ids))
        else:
            rc = lib.axon_start_nrt_profile(None, 0)
        if rc != 0:
            raise RuntimeError(f"axon_start_nrt_profile rc={rc}")
        try:
            yield
        finally:
            n = lib.axon_stop_nrt_profile(str(output_dir).encode())
            print(f"profile: {n} ntff file(s) in {output_dir}", file=sys.stderr)

    state["hook"] = _hook


def _get_nc(mask2d, has_bias):
    key = (hash(mask2d.tobytes()), has_bias)
    if key not in _CACHE:
        plans, patterns = _plan_from_mask(mask2d)
        # guard against fully-masked rows (reference maps softmax NaN -> 0)
        valid_any = (~mask2d).any(axis=1)
        guard = bool((~valid_any).any())
        _CACHE[key] = (_build(plans, len(patterns), guard, has_bias), patterns)
    return _CACHE[key]


def kernel(query, key, value, mask, Wq, bq, Wk, bk, Wv, bv, Wo, bo):
    from concourse.bass_utils import run_bass_kernel_spmd

    query = np.asarray(query, dtype=np.float32)
    key_ = np.asarray(key, dtype=np.float32)
    value = np.asarray(value, dtype=np.float32)
    mask2d = np.asarray(mask, dtype=bool).reshape(S, S)
    Wq = np.asarray(Wq, dtype=np.float32)
    Wk = np.asarray(Wk, dtype=np.float32)
    Wv = np.asarray(Wv, dtype=np.float32)
    Wo = np.asarray(Wo, dtype=np.float32)
    bq = np.asarray(bq, dtype=np.float32)
    bk = np.asarray(bk, dtype=np.float32)
    bv = np.asarray(bv, dtype=np.float32)
    bo = np.asarray(bo, dtype=np.float32)

    has_bias = (bool(bq.any()), bool(bk.any()), bool(bv.any()))
    nc, patterns = _get_nc(mask2d, has_bias)

    n_pat = len(patterns)
    if n_pat:
        mp = np.empty((CH, n_pat * CH), np.float32)
        for u, pat in enumerate(patterns):
            mp[:, u * CH:(u + 1) * CH] = pat
        mp = mp.astype(ml_dtypes.bfloat16)
    ones_row = np.ones((1, QB), ml_dtypes.bfloat16)

    in_maps = []
    for c in range(NCORES):
        b, g = divmod(c, 2)
        gsl = slice(DL * g, DL * (g + 1))
        m = {
            "xq_t": _bf16(query[b].T),
            "xk_t": _bf16(key_[b].T),
            "xv_t": _bf16(value[b].T),
            "wq_t": _bf16(Wq[gsl].T * 0.125),
            "wk_t": _bf16(Wk[gsl].T),
            "wv_t": _bf16(Wv[gsl].T),
            "wo_t": _bf16(Wo[:, gsl].T),
            "bq8": _bf16(bq[gsl].reshape(1, DL) * 0.125),
            "bk": _bf16(bk[gsl].reshape(1, DL)),
            "bv": _bf16(bv[gsl].reshape(1, DL)),
            "ones_row": ones_row,
        }
        if n_pat:
            m["maskp"] = mp
        in_maps.append(m)

    import os
    kwargs = {}
    if os.environ.get("BASS_MHA_TRACE"):
        _install_ntff_shim()
        tc_env = os.environ.get("BASS_MHA_TRACE_CORES", "0")
        cores = (list(range(NCORES)) if tc_env == "all"
                 else [int(x) for x in tc_env.split(",")])
        kwargs = dict(trace=True, trace_cores=cores)
    res = run_bass_kernel_spmd(nc, in_maps, core_ids=list(range(NCORES)), **kwargs)
    global LAST_RESULTS
    LAST_RESULTS = res

    out = np.empty((B, S, D), np.float32)
    for b in range(B):
        acc = res.results[2 * b]["outT"] + res.results[2 * b + 1]["outT"]
        out[b] = acc.T + bo[None, :]
    return out
